# revision 21
# baseline (speedup 1.0000x reference)
"""BitNet attention block on 8 TRN2 NeuronCores (tensor-parallel over heads).

Self-contained: kernel(**inputs) takes full inputs, shards internally,
runs one SPMD Bass program on cores 0-7, reassembles the full output.

Sharding: core c owns Q heads [4c,4c+4), KV head c, o_proj output dims
[512c, 512c+512). Attention is fully local per core. Cross-core comms:
two tiny AllReduces for the BitNet absmean gammas and four bf16
AllGathers (one per (batch, 512-token chunk)) of the attention output.

Key optimizations vs the fp32r baseline:
- Exact ternary {-1,0,+1} weights stored bf16; gamma scales folded into
  activation `scale=` APs (exp carries gq*gk/sqrt(HD), the V copy gv,
  the o_proj copy go). x is bf16 (host-converted). Quant decisions
  reproduce round-half-even+clip via wq = (w > g/2) - (w < -g/2).
- Everything the PE streams in the hot loops is bf16 (fp32r moving
  operands measure ~2x slower on HW than the cost model claims).
- The causal mask is added on the PE itself (an accumulating
  identity @ mask matmul into the score PSUM group) so the
  score->exp chain never crosses through the Vector engine.
- Softmax normalization: ones-broadcast matmul of the PSUM z-row,
  then a full-width [128,512] reciprocal (a [1,512] reciprocal is
  single-lane and costs 3.3us).
- Gamma AllReduce results are read back with a partition-broadcast
  DMA so thresholds are pure Vector work (no PE/Scalar hops on the
  critical path). AR2-dependent Wo work is emitted mid-phase-1.
- DMA descriptor issue costs ~0.6us, so all tile loads ride
  multi-tile slab DMAs built with AP rearrange+transpose.
- Phase 3 for batch b overlaps the other batch's AllGathers; the
  per-qc AllGather split lets agout land earlier.
"""
import os
import sys
sys.path.insert(0, "/opt/trn_rl_repo")
import numpy as np
import ml_dtypes

B, S, H = 2, 1024, 4096
NH, NKV, HD = 32, 8, 128
NCORES = 8
T = B * S
QH = NH // NCORES          # 4 q-heads per core
MSH = H // NCORES          # 512 o_proj out-dims per core
THETA = 10000.0
C_MAGIC = 12582912.0       # 1.5 * 2**23
TWO_PI = 6.283185307179586
NKT = H // 128             # 32 contraction tiles
NTC = T // 512             # 4 token chunks
SKT = S // 128             # 8 score k-tiles per batch
SQC = S // 512             # 2 q-chunks per batch
BF16NP = ml_dtypes.bfloat16

_cache = {}
last_exec_time_ns = None


def _classify_mask(mask):
    """Per (b, kt, qc) [128k x 512q] block: 0 no-op, 1 fully masked
    (skipped), 2 needs a mask add (index into deduped distinct blocks)."""
    status = np.empty((B, SKT, SQC), dtype=np.int8)
    blk_idx = {}
    distinct = []
    seen = {}
    for b in range(B):
        mb = np.asarray(mask[b, 0], dtype=np.float32)
        for kt in range(SKT):
            for qc in range(SQC):
                blk = mb[qc * 512:(qc + 1) * 512, kt * 128:(kt + 1) * 128]
                if not blk.any():
                    status[b, kt, qc] = 0
                elif (blk <= -1e4).all():
                    status[b, kt, qc] = 1
                else:
                    status[b, kt, qc] = 2
                    kb = blk.tobytes()
                    if kb not in seen:
                        seen[kb] = len(distinct)
                        distinct.append(np.ascontiguousarray(blk.T))
                    blk_idx[(b, kt, qc)] = seen[kb]
    if distinct:
        packed = np.concatenate(distinct, axis=0)
    else:
        packed = np.zeros((128, 512), dtype=np.float32)
    return status, blk_idx, np.ascontiguousarray(packed.astype(BF16NP))


def _cody_consts():
    c1 = float(np.float32(6.28125))
    r = np.float64(TWO_PI) - c1
    c2 = float(np.float32(r - np.remainder(r, 2.0 ** -24)))
    c3 = float(np.float32(np.float64(TWO_PI) - c1 - float(c2)))
    return c1, c2, c3


def _build(status, blk_idx, n_blk):
    from concourse import bacc, tile, mybir

    F32 = mybir.dt.float32
    F32R = mybir.dt.float32r
    BF = mybir.dt.bfloat16
    ACTF = mybir.ActivationFunctionType
    ALU = mybir.AluOpType
    X = mybir.AxisListType.X
    RG = [list(range(NCORES))]
    c1, c2, c3 = _cody_consts()

    nc = bacc.Bacc("TRN2", target_bir_lowering=False, debug=False,
                   num_devices=NCORES)

    xT = nc.dram_tensor("xT", [H, T], BF, kind="ExternalInput")
    wqT = nc.dram_tensor("wqT", [H, QH * HD], F32, kind="ExternalInput")
    wkT = nc.dram_tensor("wkT", [H, HD], F32, kind="ExternalInput")
    wvT = nc.dram_tensor("wvT", [H, HD], F32, kind="ExternalInput")
    woT = nc.dram_tensor("woT", [H, MSH], F32, kind="ExternalInput")
    maskP = nc.dram_tensor("maskP", [n_blk * 128, 512], BF,
                           kind="ExternalInput")
    pos = nc.dram_tensor("pos", [1, T], F32, kind="ExternalInput")
    outN = nc.dram_tensor("outN", [T, MSH], F32, kind="ExternalOutput")

    idnb_c = nc.inline_tensor(np.eye(128, dtype=BF16NP), name="idnb_c")
    onesm_c = nc.inline_tensor(np.ones((1, 128), np.float32), name="onesm_c")
    onesk_c = nc.inline_tensor(np.ones((128, 1), np.float32), name="onesk_c")
    invf_np = (1.0 / THETA ** (np.arange(0, HD, 2, dtype=np.float32) / HD))
    invf_np = np.concatenate([invf_np, invf_np]).reshape(HD, 1)
    invf_c = nc.inline_tensor(invf_np.astype(np.float32), name="invf_c")

    NQ = float(NH * HD * H)
    NK = float(NKV * HD * H)
    NO = float(H * NH * HD)
    ISQ = float(1.0 / np.sqrt(HD))

    def tiled(src):
        """[(i 128), c] DRAM slice -> [128, i, c] AP (partition-major)."""
        return src.rearrange("(i p) c -> i p c", p=128).transpose([1, 0, 2])

    with tile.TileContext(nc) as tc, \
         nc.allow_low_precision(reason="bf16 ternary kernel"):
        with tc.tile_pool(name="cpool", bufs=1) as cpool, \
             tc.tile_pool(name="dbounce", bufs=1, space="DRAM") as dbounce:
            # DRAM bounce tiles for the collectives
            arq_in = dbounce.tile([1, 8], F32, name="arq_in")
            arq_out = dbounce.tile([1, 8], F32, name="arq_out",
                                   addr_space="Shared")
            aro_in = dbounce.tile([1, 8], F32, name="aro_in")
            aro_out = dbounce.tile([1, 8], F32, name="aro_out",
                                   addr_space="Shared")
            agin = [[dbounce.tile([QH * HD, 512], BF, name=f"agi{b}{qc}")
                     for qc in range(SQC)] for b in range(B)]
            agout = [[dbounce.tile([H, 512], BF, name=f"ago{b}{qc}",
                                   addr_space="Shared")
                      for qc in range(SQC)] for b in range(B)]

            # constants
            idnb = cpool.tile([128, 128], BF, name="idnb")
            nc.sync.dma_start(out=idnb[:], in_=idnb_c[:, :])
            oneskb = cpool.tile([128, 1], BF, name="oneskb")
            nc.vector.memset(oneskb[:], 1.0)
            onesk = cpool.tile([128, 1], F32, name="onesk")
            nc.sync.dma_start(out=onesk[:], in_=onesk_c[:, :])
            onesmr = cpool.tile([1, 128], F32R, name="onesmr")
            nc.sync.dma_start(out=onesmr[:], in_=onesm_c[:, :].bitcast(F32R))
            invf = cpool.tile([128, 1], F32, name="invf")
            nc.sync.dma_start(out=invf[:], in_=invf_c[:, :])
            mask_sb = cpool.tile([128, n_blk * 512], BF, name="mask_sb")
            nc.sync.dma_start(out=mask_sb[:], in_=tiled(maskP[:, :]))
            # broadcast scalars: 0 thq 1 thqn 2 thk 3 thkn 4 thv 5 thvn
            #                    6 cqk 7 gv | 8 tho 9 thon 10 go
            bsc = cpool.tile([128, 12], F32, name="bsc")
            gw = cpool.tile([128, 4], F32, name="gw")
            arq128 = cpool.tile([128, 8], F32, name="arq128")
            aro128 = cpool.tile([128, 8], F32, name="aro128")

            with tc.tile_pool(name="tab", bufs=1) as tab:
                cos_sb = tab.tile([128, T], F32, name="cos_sb")
                ss_sb = tab.tile([128, T], F32, name="ss_sb")
                # RoPE tables: Cody-Waite range reduction + Sin (emitted
                # first so they run during the initial weight DMA).
                with tc.tile_pool(name="rtab", bufs=3) as rtab:
                    for tcn in range(NTC):
                        cs = slice(tcn * 512, (tcn + 1) * 512)
                        pf = rtab.tile([128, 512], F32, name=f"pf{tcn}",
                                       tag="pf")
                        nc.scalar.dma_start(
                            out=pf[:],
                            in_=pos[0:1, cs].partition_broadcast(128))
                        f_sb = rtab.tile([128, 512], F32, name=f"f{tcn}",
                                         tag="f")
                        nc.scalar.activation(f_sb[:], pf[:], ACTF.Copy,
                                             scale=invf[:])
                        k_sb = rtab.tile([128, 512], F32, name=f"kk{tcn}",
                                         tag="kk")
                        nc.vector.tensor_scalar(k_sb[:], f_sb[:],
                                                1.0 / TWO_PI, C_MAGIC,
                                                ALU.mult, ALU.add)
                        nc.vector.tensor_scalar(k_sb[:], k_sb[:], C_MAGIC,
                                                None, ALU.subtract)
                        y_sb = rtab.tile([128, 512], F32, name=f"y{tcn}",
                                         tag="y")
                        nc.vector.scalar_tensor_tensor(
                            y_sb[:], k_sb[:], -c1, f_sb[:], ALU.mult,
                            ALU.add)
                        nc.vector.scalar_tensor_tensor(
                            y_sb[:], k_sb[:], -c2, y_sb[:], ALU.mult,
                            ALU.add)
                        nc.vector.scalar_tensor_tensor(
                            y_sb[:], k_sb[:], -c3, y_sb[:], ALU.mult,
                            ALU.add)
                        nc.scalar.activation(ss_sb[0:64, cs], y_sb[0:64, :],
                                             ACTF.Sin, scale=-1.0)
                        nc.scalar.activation(ss_sb[64:128, cs],
                                             y_sb[64:128, :], ACTF.Sin)
                        yc = rtab.tile([128, 512], F32, name=f"yc{tcn}",
                                       tag="yc")
                        nc.vector.tensor_scalar(yc[:], y_sb[:],
                                                float(np.pi / 2), None,
                                                ALU.add)
                        m_sb = rtab.tile([128, 512], F32, name=f"mm{tcn}",
                                         tag="mm")
                        nc.vector.tensor_scalar(m_sb[:], yc[:],
                                                float(np.pi), None,
                                                ALU.is_gt)
                        nc.vector.scalar_tensor_tensor(
                            yc[:], m_sb[:], -TWO_PI, yc[:], ALU.mult,
                            ALU.add)
                        nc.scalar.activation(cos_sb[:, cs], yc[:], ACTF.Sin)

                # ---- pools ordered by lifetime for LIFO release ----
                gacc = tc.alloc_tile_pool(name="gacc", bufs=1)
                w3 = tc.alloc_tile_pool(name="w3", bufs=1)
                wo_sb = w3.tile([128, NKT * MSH], BF, name="wo_sb")
                qkvp = tc.alloc_tile_pool(name="qkv", bufs=1)
                qT_sb = [qkvp.tile([128, T], BF, name=f"qT{h}")
                         for h in range(QH)]
                kT_sb = qkvp.tile([128, T], BF, name="kT_sb")
                vT_sb = qkvp.tile([128, T], BF, name="vT_sb")
                vnatp = tc.alloc_tile_pool(name="vnatp", bufs=2)
                vnats = []
                wop2 = tc.alloc_tile_pool(name="wop2", bufs=3)
                qsc2 = tc.alloc_tile_pool(name="qsc2", bufs=2)
                wbig = tc.alloc_tile_pool(name="wbig", bufs=1)
                wq_sb = wbig.tile([128, NKT * 512], BF, name="wq_sb")
                wbig2 = tc.alloc_tile_pool(name="wbig2", bufs=1)
                wk_sb = wbig2.tile([128, NKT * 128], BF, name="wk_sb")
                wv_sb = wbig2.tile([128, NKT * 128], BF, name="wv_sb")

                # ---- gamma prepass: Wq/Wo streamed, Wk/Wv staged ----
                wstage = tc.alloc_tile_pool(name="wstage", bufs=1)
                wk_f = wstage.tile([128, NKT * 128], F32, name="wk_f")

                accq = gacc.tile([128, NKT], F32, name="accq")
                acck = gacc.tile([128, NKT], F32, name="acck")
                accv = gacc.tile([128, NKT], F32, name="accv")
                acco = gacc.tile([128, NKT], F32, name="acco")
                g4 = gacc.tile([128, 4], F32, name="g4")

                with tc.tile_pool(name="wqpre", bufs=2) as wqpre, \
                     tc.tile_pool(name="wopre", bufs=3) as wopre:
                    for j in range(16):
                        sl = wqpre.tile([128, 2 * 512], F32, name=f"wqp{j}",
                                        tag="wqp")
                        nc.sync.dma_start(
                            out=sl[:],
                            in_=tiled(wqT[j * 256:(j + 1) * 256, :]))
                        for i in range(2):
                            nc.vector.tensor_reduce(
                                accq[:, j * 2 + i:j * 2 + i + 1],
                                sl[:, i * 512:(i + 1) * 512], X, ALU.add,
                                apply_absolute_value=True)
                    nc.sync.dma_start(out=wk_f[:], in_=tiled(wkT[:, :]))
                    for i in range(NKT):
                        nc.vector.tensor_reduce(
                            acck[:, i:i + 1],
                            wk_f[:, i * 128:(i + 1) * 128], X, ALU.add,
                            apply_absolute_value=True)
                    for j in range(4):
                        sl = wqpre.tile([128, 8 * 128], F32,
                                        name=f"wvp{j}", tag="wvp")
                        nc.sync.dma_start(
                            out=sl[:],
                            in_=tiled(wvT[j * 1024:(j + 1) * 1024, :]))
                        for i in range(8):
                            nc.vector.tensor_reduce(
                                accv[:, j * 8 + i:j * 8 + i + 1],
                                sl[:, i * 128:(i + 1) * 128], X, ALU.add,
                                apply_absolute_value=True)
                    nc.vector.tensor_reduce(g4[:, 0:1], accq[:], X, ALU.add)
                    nc.vector.tensor_reduce(g4[:, 1:2], acck[:], X, ALU.add)
                    nc.vector.tensor_reduce(g4[:, 2:3], accv[:], X, ALU.add)
                    # Wo |.| sums ride along behind the q/k/v ones
                    for j in range(16):
                        sl = wopre.tile([128, 2 * 512], F32, name=f"wop{j}",
                                        tag="wop")
                        nc.gpsimd.dma_start(
                            out=sl[:],
                            in_=tiled(woT[j * 256:(j + 1) * 256, :]))
                        for i in range(2):
                            nc.vector.tensor_reduce(
                                acco[:, j * 2 + i:j * 2 + i + 1],
                                sl[:, i * 512:(i + 1) * 512], X, ALU.add,
                                apply_absolute_value=True)
                    nc.vector.tensor_reduce(g4[:, 3:4], acco[:], X, ALU.add)

                with tc.tile_pool(name="pgam", bufs=1, space="PSUM") \
                        as pgam:
                    # AllReduce #1: q/k/v gamma sums
                    pg_q = pgam.tile([1, 3], F32, name="pg_q", tag="pg")
                    nc.tensor.matmul(pg_q[:], onesk[:], g4[:, 0:3],
                                     start=True, stop=True)
                    gq_sb = gacc.tile([1, 8], F32, name="gq_sb")
                    nc.vector.memset(gq_sb[:], 0.0)
                    nc.scalar.copy(gq_sb[:, 0:3], pg_q[:])
                    nc.sync.dma_start(out=arq_in[:], in_=gq_sb[:])
                    nc.gpsimd.collective_compute(
                        "AllReduce", ALU.add, replica_groups=RG,
                        ins=[arq_in[:].opt()], outs=[arq_out[:].opt()])

                    # AllReduce #2: Wo gamma sum (issued right behind #1;
                    # result consumed mid-phase-1)
                    pg_o = pgam.tile([1, 1], F32, name="pg_o", tag="pg")
                    nc.tensor.matmul(pg_o[:], onesk[:], g4[:, 3:4],
                                     start=True, stop=True)
                    go_sb = gacc.tile([1, 8], F32, name="go_sb")
                    nc.vector.memset(go_sb[:], 0.0)
                    nc.scalar.copy(go_sb[:, 0:1], pg_o[:])
                    nc.scalar.dma_start(out=aro_in[:], in_=go_sb[:])
                    nc.gpsimd.collective_compute(
                        "AllReduce", ALU.add, replica_groups=RG,
                        ins=[aro_in[:].opt()], outs=[aro_out[:].opt()])

                # partition-broadcast readback: thresholds become pure
                # Vector work, no PE/Scalar hops before quant can start
                nc.scalar.dma_start(
                    out=arq128[:],
                    in_=arq_out[:, :].partition_broadcast(128))
                nc.vector.tensor_scalar(gw[:, 0:1], arq128[:, 0:1],
                                        1.0 / NQ, 1e-5, ALU.mult, ALU.add)
                nc.vector.tensor_scalar(gw[:, 1:3], arq128[:, 1:3],
                                        1.0 / NK, 1e-5, ALU.mult, ALU.add)
                nc.vector.tensor_scalar(bsc[:, 0:1], gw[:, 0:1], 0.5,
                                        None, ALU.mult)
                nc.vector.tensor_scalar(bsc[:, 1:2], gw[:, 0:1], -0.5,
                                        None, ALU.mult)
                nc.vector.tensor_scalar(bsc[:, 2:3], gw[:, 1:2], 0.5,
                                        None, ALU.mult)
                nc.vector.tensor_scalar(bsc[:, 3:4], gw[:, 1:2], -0.5,
                                        None, ALU.mult)
                nc.vector.tensor_scalar(bsc[:, 4:5], gw[:, 2:3], 0.5,
                                        None, ALU.mult)
                nc.vector.tensor_scalar(bsc[:, 5:6], gw[:, 2:3], -0.5,
                                        None, ALU.mult)
                nc.vector.tensor_mul(bsc[:, 6:7], gw[:, 0:1], gw[:, 1:2])
                nc.vector.tensor_scalar(bsc[:, 6:7], bsc[:, 6:7], ISQ,
                                        None, ALU.mult)
                nc.vector.tensor_copy(bsc[:, 7:8], gw[:, 2:3])

                THQ, THQN = bsc[:, 0:1], bsc[:, 1:2]
                THK, THKN = bsc[:, 2:3], bsc[:, 3:4]
                THV, THVN = bsc[:, 4:5], bsc[:, 5:6]
                CQK, GV = bsc[:, 6:7], bsc[:, 7:8]
                THO, THON, GO = bsc[:, 8:9], bsc[:, 9:10], bsc[:, 10:11]

                def quant_tile(pool, src, dst, thp, thn, tg):
                    scr = pool.tile([128, src.shape[1]], F32,
                                    name=f"qs_{tg}", tag=f"qs{tg[0]}")
                    nc.vector.tensor_scalar(scr[:], src, thn, None,
                                            ALU.is_lt)
                    nc.vector.scalar_tensor_tensor(
                        dst, src, thp, scr[:], ALU.is_gt, ALU.subtract)

                # ---- quant q/k/v -> exact ternary bf16 (Wq re-read in
                # slabs that prefetch during the AllReduce wait)
                with tc.tile_pool(name="wqst", bufs=2) as wqst, \
                     tc.tile_pool(name="wvst", bufs=4) as wvst, \
                     tc.tile_pool(name="qscr", bufs=2) as qscr:
                    wv2 = []
                    for j in range(4):
                        sl = wvst.tile([128, 8 * 128], F32,
                                       name=f"wv2_{j}", tag="wv2")
                        nc.sync.dma_start(
                            out=sl[:],
                            in_=tiled(wvT[j * 1024:(j + 1) * 1024, :]))
                        wv2.append(sl)
                    for j in range(16):
                        sl = wqst.tile([128, 2 * 512], F32,
                                       name=f"wq2_{j}", tag="wq2")
                        nc.sync.dma_start(
                            out=sl[:],
                            in_=tiled(wqT[j * 256:(j + 1) * 256, :]))
                        for i in range(2):
                            k = j * 2 + i
                            quant_tile(qscr, sl[:, i * 512:(i + 1) * 512],
                                       wq_sb[:, k * 512:(k + 1) * 512],
                                       THQ, THQN, f"q{k}")
                            quant_tile(qscr,
                                       wk_f[:, k * 128:(k + 1) * 128],
                                       wk_sb[:, k * 128:(k + 1) * 128],
                                       THK, THKN, f"k{k}")
                            quant_tile(qscr,
                                       wv2[k // 8][:,
                                                   (k % 8) * 128:
                                                   (k % 8 + 1) * 128],
                                       wv_sb[:, k * 128:(k + 1) * 128],
                                       THV, THVN, f"v{k}")
                wstage.release()

                # ---- phase 1: QKV projections + RoPE + Wo quant ----
                with tc.tile_pool(name="xin", bufs=4) as xin, \
                     tc.tile_pool(name="rope", bufs=2) as rope, \
                     tc.tile_pool(name="p1", bufs=8, space="PSUM") as p1:
                    for tcn in range(NTC):
                        cs = slice(tcn * 512, (tcn + 1) * 512)
                        xsl = []
                        for j in range(4):
                            sl = xin.tile([128, 8 * 512], BF,
                                          name=f"x{tcn}_{j}", tag="xt")
                            nc.sync.dma_start(
                                out=sl[:],
                                in_=tiled(xT[j * 1024:(j + 1) * 1024, cs]))
                            xsl.append(sl)
                        pq = [p1.tile([128, 512], F32, name=f"pq{tcn}_{h}",
                                      tag="p1") for h in range(QH)]
                        pk = p1.tile([128, 512], F32, name=f"pk{tcn}",
                                     tag="p1")
                        pv = p1.tile([128, 512], F32, name=f"pv{tcn}",
                                     tag="p1")
                        for kt in range(NKT):
                            xt_ = xsl[kt // 8][:, (kt % 8) * 512:
                                               (kt % 8 + 1) * 512]
                            st, sp = (kt == 0), (kt == NKT - 1)
                            for h in range(QH):
                                nc.tensor.matmul(
                                    pq[h][:],
                                    wq_sb[:, kt * 512 + h * 128:
                                          kt * 512 + (h + 1) * 128],
                                    xt_, start=st, stop=sp,
                                    skip_group_check=True)
                            nc.tensor.matmul(
                                pk[:], wk_sb[:, kt * 128:(kt + 1) * 128],
                                xt_, start=st, stop=sp,
                                skip_group_check=True)
                            nc.tensor.matmul(
                                pv[:], wv_sb[:, kt * 128:(kt + 1) * 128],
                                xt_, start=st, stop=sp,
                                skip_group_check=True)

                        def rope_apply(psrc, dst_ap, tg):
                            m1 = rope.tile([128, 512], F32, name=f"m1{tg}",
                                           tag="m1")
                            nc.vector.tensor_mul(m1[:], psrc[:],
                                                 cos_sb[:, cs])
                            m2 = rope.tile([128, 512], F32, name=f"m2{tg}",
                                           tag="m2")
                            nc.vector.tensor_mul(m2[0:64, :],
                                                 psrc[64:128, :],
                                                 ss_sb[0:64, cs])
                            nc.vector.tensor_mul(m2[64:128, :],
                                                 psrc[0:64, :],
                                                 ss_sb[64:128, cs])
                            nc.vector.tensor_add(dst_ap, m1[:], m2[:])

                        for h in range(QH):
                            rope_apply(pq[h], qT_sb[h][:, cs], f"_{tcn}_{h}")
                        rope_apply(pk, kT_sb[:, cs], f"k_{tcn}")
                        nc.scalar.activation(vT_sb[:, cs], pv[:], ACTF.Copy,
                                             scale=GV)

                        # vnat transposes ride along once their vT
                        # chunks exist (b0 after tcn1, b1 after tcn3)
                        if tcn % 2 == 1:
                            b = tcn // 2
                            vnat = vnatp.tile([128, S], BF,
                                              name=f"vnat{b}", tag="vnat")
                            vnats.append(vnat)
                            for kt in range(SKT):
                                ptr = p1.tile([128, 512], BF,
                                              name=f"ptr{b}_{kt}",
                                              tag="p1")
                                nc.tensor.transpose(
                                    ptr[:, 0:128],
                                    vT_sb[:, b * S + kt * 128:
                                          b * S + (kt + 1) * 128],
                                    idnb[:])
                                nc.vector.tensor_copy(
                                    vnat[:, kt * 128:(kt + 1) * 128],
                                    ptr[:, 0:128])

                wbig2.release()
                wbig.release()

                # ---- phase 2: attention (all-bf16 PE path) ----
                with tc.tile_pool(name="epool", bufs=8) as epool, \
                     tc.tile_pool(name="aop", bufs=4) as aop, \
                     tc.tile_pool(name="zpool", bufs=2) as zpool, \
                     tc.tile_pool(name="ps_s", bufs=4,
                                  space="PSUM") as ps_s, \
                     tc.tile_pool(name="ps_o", bufs=2,
                                  space="PSUM") as ps_o, \
                     tc.tile_pool(name="ps_x", bufs=1,
                                  space="PSUM") as ps_x:
                    # Wo thresholds (AR2 finished long ago) + slab DMAs;
                    # the quant compares interleave with attention below
                    nc.scalar.dma_start(
                        out=aro128[:],
                        in_=aro_out[:, :].partition_broadcast(128))
                    nc.vector.tensor_scalar(gw[:, 3:4], aro128[:, 0:1],
                                            1.0 / NO, 1e-5, ALU.mult,
                                            ALU.add)
                    nc.vector.tensor_scalar(bsc[:, 8:9], gw[:, 3:4], 0.5,
                                            None, ALU.mult)
                    nc.vector.tensor_scalar(bsc[:, 9:10], gw[:, 3:4], -0.5,
                                            None, ALU.mult)
                    nc.vector.tensor_copy(bsc[:, 10:11], gw[:, 3:4])
                    wosl = []

                    def wo_slab_fetch():
                        j = len(wosl)
                        if j >= 8:
                            return
                        wt = wop2.tile([128, 4 * 512], F32,
                                       name=f"wo2_{j}", tag="wo2")
                        nc.gpsimd.dma_start(
                            out=wt[:],
                            in_=tiled(woT[j * 512:(j + 1) * 512, :]))
                        wosl.append(wt)

                    wo_slab_fetch()
                    wo_slab_fetch()
                    woq_state = [0]

                    def wo_quant_some(n):
                        for _ in range(n):
                            k = woq_state[0]
                            if k >= NKT:
                                return
                            woq_state[0] += 1
                            if k % 4 == 0:
                                wo_slab_fetch()
                            quant_tile(
                                qsc2,
                                wosl[k // 4][:, (k % 4) * 512:
                                             (k % 4 + 1) * 512],
                                wo_sb[:, k * MSH:(k + 1) * MSH],
                                THO, THON, f"o{k}")

                    for b in range(B):
                        boff = b * S
                        vnat = vnats[b]
                        for qc in range(SQC):
                            kts = [kt for kt in range(SKT)
                                   if status[b, kt, qc] != 1]
                            assert kts, "fully-masked softmax row"
                            for h in range(QH):
                                qsl = qT_sb[h][:, boff + qc * 512:
                                               boff + (qc + 1) * 512]
                                # scores stream on the PE; the causal mask
                                # is added by an accumulating idn @ mask
                                # matmul (no Vector hop in the chain)
                                es = []
                                for kt in kts:
                                    masked = status[b, kt, qc] == 2
                                    ps_ = ps_s.tile([128, 512], F32,
                                                    name=f"s{b}{h}{qc}{kt}",
                                                    tag="ps")
                                    nc.tensor.matmul(
                                        ps_[:],
                                        kT_sb[:, boff + kt * 128:
                                              boff + (kt + 1) * 128],
                                        qsl, start=True, stop=not masked,
                                        skip_group_check=True)
                                    if masked:
                                        mi = blk_idx[(b, kt, qc)]
                                        nc.tensor.matmul(
                                            ps_[:], idnb[:],
                                            mask_sb[:, mi * 512:
                                                    (mi + 1) * 512],
                                            start=False, stop=True,
                                            skip_group_check=True)
                                    e = epool.tile([128, 512], BF,
                                                   name=f"e{b}{h}{qc}{kt}",
                                                   tag="e")
                                    nc.scalar.activation(e[:], ps_[:],
                                                         ACTF.Exp,
                                                         scale=CQK)
                                    es.append(e)
                                pz = ps_x.tile([1, 512], F32,
                                               name=f"pz{b}{h}{qc}",
                                               tag="pz")
                                po = ps_o.tile([128, 512], F32,
                                               name=f"po{b}{h}{qc}",
                                               tag="po")
                                for i, kt in enumerate(kts):
                                    fst = (i == 0)
                                    lst = (i == len(kts) - 1)
                                    nc.tensor.matmul(
                                        pz[:], oneskb[:], es[i][:],
                                        start=fst, stop=lst,
                                        skip_group_check=True)
                                    nc.tensor.matmul(
                                        po[:],
                                        vnat[:, kt * 128:(kt + 1) * 128],
                                        es[i][:], start=fst, stop=lst,
                                        skip_group_check=True)
                                # z -> SBUF, ones-broadcast, full-width
                                # reciprocal (a [1,512] recip is 1-lane)
                                zsb = zpool.tile([1, 512], F32R,
                                                 name=f"zs{b}{h}{qc}",
                                                 tag="zs")
                                nc.scalar.copy(zsb[:], pz[:])
                                pzb = ps_x.tile([128, 512], F32,
                                                name=f"pzb{b}{h}{qc}",
                                                tag="pzb")
                                nc.tensor.matmul(pzb[:], onesmr[:], zsb[:],
                                                 start=True, stop=True,
                                                 skip_group_check=True)
                                zb = zpool.tile([128, 512], F32,
                                                name=f"zb{b}{h}{qc}",
                                                tag="zb")
                                nc.vector.reciprocal(zb[:], pzb[:])
                                ao = aop.tile([128, 512], BF,
                                              name=f"ao{b}{h}{qc}",
                                              tag="ao")
                                nc.vector.tensor_mul(ao[:], po[:], zb[:])
                                nc.sync.dma_start(
                                    out=agin[b][qc][h * 128:(h + 1) * 128,
                                                    :],
                                    in_=ao[:])
                                wo_quant_some(3)
                            nc.gpsimd.collective_compute(
                                "AllGather", ALU.bypass, replica_groups=RG,
                                ins=[agin[b][qc][:].opt()],
                                outs=[agout[b][qc][:].opt()])

                qsc2.release()
                wop2.release()

                # ---- phase 3: o_proj, per (batch, qchunk) for overlap ----
                with tc.tile_pool(name="a3", bufs=2) as a3, \
                     tc.tile_pool(name="o3", bufs=2) as o3, \
                     tc.tile_pool(name="p3", bufs=4, space="PSUM") as p3:
                    for ch in range(NTC):
                        b, q2 = ch // 2, ch % 2
                        at = a3.tile([128, NKT * 512], BF, name=f"at{ch}",
                                     tag="at")
                        nc.sync.dma_start(out=at[:],
                                          in_=tiled(agout[b][q2][:, :]))
                        for tt in range(4):
                            pout = p3.tile([128, 512], F32,
                                           name=f"po3_{ch}{tt}", tag="pout")
                            for kt in range(NKT):
                                nc.tensor.matmul(
                                    pout[:],
                                    at[:, kt * 512 + tt * 128:
                                       kt * 512 + (tt + 1) * 128],
                                    wo_sb[:, kt * MSH:(kt + 1) * MSH],
                                    start=(kt == 0), stop=(kt == NKT - 1),
                                    skip_group_check=True)
                            osb = o3.tile([128, 512], F32,
                                          name=f"osb{ch}{tt}", tag="osb")
                            nc.scalar.activation(osb[:], pout[:], ACTF.Copy,
                                                 scale=GO)
                            nc.sync.dma_start(
                                out=outN[ch * 512 + tt * 128:
                                         ch * 512 + (tt + 1) * 128, :],
                                in_=osb[:])
                vnatp.release()
                qkvp.release()
                w3.release()
                gacc.release()

    nc.compile()
    return nc


def kernel(hidden_states, Wq, Wk, Wv, Wo, attention_mask, position_ids):
    from concourse.bass_utils import run_bass_kernel_spmd
    from concourse.bass_interp import get_hw_module

    hs = np.ascontiguousarray(np.asarray(hidden_states, dtype=np.float32))
    Wq = np.asarray(Wq, dtype=np.float32)
    Wk = np.asarray(Wk, dtype=np.float32)
    Wv = np.asarray(Wv, dtype=np.float32)
    Wo = np.asarray(Wo, dtype=np.float32)
    mask = np.asarray(attention_mask, dtype=np.float32)
    posf = np.ascontiguousarray(
        np.asarray(position_ids).reshape(1, T).astype(np.float32))

    status, blk_idx, packed = _classify_mask(mask)
    n_blk = packed.shape[0] // 128
    assert n_blk <= 16, "too many distinct mask blocks"

    key = (status.tobytes(), tuple(sorted(blk_idx.items())), n_blk)
    if key not in _cache:
        nc = _build(status, blk_idx, n_blk)
        nc.m = get_hw_module(nc.m)
        _cache[key] = nc
    nc = _cache[key]

    xT = np.ascontiguousarray(hs.reshape(T, H).T.astype(BF16NP))
    in_maps = []
    for c in range(NCORES):
        in_maps.append({
            "xT": xT,
            "wqT": np.ascontiguousarray(
                Wq[c * QH * HD:(c + 1) * QH * HD, :].T),
            "wkT": np.ascontiguousarray(Wk[c * HD:(c + 1) * HD, :].T),
            "wvT": np.ascontiguousarray(Wv[c * HD:(c + 1) * HD, :].T),
            "woT": np.ascontiguousarray(Wo[c * MSH:(c + 1) * MSH, :].T),
            "maskP": packed,
            "pos": posf,
        })
    res = run_bass_kernel_spmd(nc, in_maps, core_ids=list(range(NCORES)),
                               trace=bool(os.environ.get("BITNET_TRACE")))
    global last_exec_time_ns
    last_exec_time_ns = res.exec_time_ns
    out = np.concatenate(
        [res.results[c]["outN"] for c in range(NCORES)], axis=1)  # (T, MSH*8)
    return np.ascontiguousarray(out).reshape(B, S, H).astype(np.float32)


# revision 23
# speedup vs baseline: 1.0279x; 1.0279x over previous
"""BitNet attention block on 8 TRN2 NeuronCores (tensor-parallel over heads).

Self-contained: kernel(**inputs) takes full inputs, shards internally,
runs one SPMD Bass program on cores 0-7, reassembles the full output.

Sharding: core c owns Q heads [4c,4c+4), KV head c, o_proj output dims
[512c, 512c+512). Attention is fully local per core. Cross-core comms:
two tiny AllReduces for the BitNet absmean gammas and four bf16
AllGathers (one per (batch, 512-token chunk)) of the attention output.

Key optimizations vs the fp32r baseline:
- Exact ternary {-1,0,+1} weights stored bf16; gamma scales folded into
  activation `scale=` APs (exp carries gq*gk/sqrt(HD), the V copy gv,
  the o_proj copy go). x is bf16 (host-converted). Quant decisions
  reproduce round-half-even+clip via wq = (w > g/2) - (w < -g/2).
- Everything the PE streams in the hot loops is bf16 (fp32r moving
  operands measure ~2x slower on HW than the cost model claims).
- The causal mask is added on the PE itself (an accumulating
  identity @ mask matmul into the score PSUM group) so the
  score->exp chain never crosses through the Vector engine.
- Softmax normalization: ones-broadcast matmul of the PSUM z-row,
  then a full-width [128,512] reciprocal (a [1,512] reciprocal is
  single-lane and costs 3.3us).
- Gamma AllReduce results are read back with a partition-broadcast
  DMA so thresholds are pure Vector work (no PE/Scalar hops on the
  critical path). AR2-dependent Wo work is emitted mid-phase-1.
- DMA descriptor issue costs ~0.6us, so all tile loads ride
  multi-tile slab DMAs built with AP rearrange+transpose.
- Phase 3 for batch b overlaps the other batch's AllGathers; the
  per-qc AllGather split lets agout land earlier.
"""
import os
import sys
sys.path.insert(0, "/opt/trn_rl_repo")
import numpy as np
import ml_dtypes

B, S, H = 2, 1024, 4096
NH, NKV, HD = 32, 8, 128
NCORES = 8
T = B * S
QH = NH // NCORES          # 4 q-heads per core
MSH = H // NCORES          # 512 o_proj out-dims per core
THETA = 10000.0
C_MAGIC = 12582912.0       # 1.5 * 2**23
TWO_PI = 6.283185307179586
NKT = H // 128             # 32 contraction tiles
NTC = T // 512             # 4 token chunks
SKT = S // 128             # 8 score k-tiles per batch
SQC = S // 512             # 2 q-chunks per batch
BF16NP = ml_dtypes.bfloat16

_cache = {}
last_exec_time_ns = None


def _classify_mask(mask):
    """Per (b, kt, qc) [128k x 512q] block: 0 no-op, 1 fully masked
    (skipped), 2 needs a mask add (index into deduped distinct blocks)."""
    status = np.empty((B, SKT, SQC), dtype=np.int8)
    blk_idx = {}
    distinct = []
    seen = {}
    for b in range(B):
        mb = np.asarray(mask[b, 0], dtype=np.float32)
        for kt in range(SKT):
            for qc in range(SQC):
                blk = mb[qc * 512:(qc + 1) * 512, kt * 128:(kt + 1) * 128]
                if not blk.any():
                    status[b, kt, qc] = 0
                elif (blk <= -1e4).all():
                    status[b, kt, qc] = 1
                else:
                    status[b, kt, qc] = 2
                    kb = blk.tobytes()
                    if kb not in seen:
                        seen[kb] = len(distinct)
                        distinct.append(np.ascontiguousarray(blk.T))
                    blk_idx[(b, kt, qc)] = seen[kb]
    if distinct:
        packed = np.concatenate(distinct, axis=0)
    else:
        packed = np.zeros((128, 512), dtype=np.float32)
    return status, blk_idx, np.ascontiguousarray(packed.astype(BF16NP))


def _cody_consts():
    c1 = float(np.float32(6.28125))
    r = np.float64(TWO_PI) - c1
    c2 = float(np.float32(r - np.remainder(r, 2.0 ** -24)))
    c3 = float(np.float32(np.float64(TWO_PI) - c1 - float(c2)))
    return c1, c2, c3


def _build(status, blk_idx, n_blk):
    from concourse import bacc, tile, mybir

    F32 = mybir.dt.float32
    F32R = mybir.dt.float32r
    BF = mybir.dt.bfloat16
    ACTF = mybir.ActivationFunctionType
    ALU = mybir.AluOpType
    X = mybir.AxisListType.X
    RG = [list(range(NCORES))]
    c1, c2, c3 = _cody_consts()

    nc = bacc.Bacc("TRN2", target_bir_lowering=False, debug=False,
                   num_devices=NCORES)

    xT = nc.dram_tensor("xT", [H, T], BF, kind="ExternalInput")
    wqT = nc.dram_tensor("wqT", [H, QH * HD], F32, kind="ExternalInput")
    wkT = nc.dram_tensor("wkT", [H, HD], F32, kind="ExternalInput")
    wvT = nc.dram_tensor("wvT", [H, HD], F32, kind="ExternalInput")
    woT = nc.dram_tensor("woT", [H, MSH], F32, kind="ExternalInput")
    maskP = nc.dram_tensor("maskP", [n_blk * 128, 512], BF,
                           kind="ExternalInput")
    pos = nc.dram_tensor("pos", [1, T], F32, kind="ExternalInput")
    outN = nc.dram_tensor("outN", [T, MSH], F32, kind="ExternalOutput")

    idnb_c = nc.inline_tensor(np.eye(128, dtype=BF16NP), name="idnb_c")
    onesm_c = nc.inline_tensor(np.ones((1, 128), np.float32), name="onesm_c")
    onesk_c = nc.inline_tensor(np.ones((128, 1), np.float32), name="onesk_c")
    invf_np = (1.0 / THETA ** (np.arange(0, HD, 2, dtype=np.float32) / HD))
    invf_np = np.concatenate([invf_np, invf_np]).reshape(HD, 1)
    invf_c = nc.inline_tensor(invf_np.astype(np.float32), name="invf_c")

    NQ = float(NH * HD * H)
    NK = float(NKV * HD * H)
    NO = float(H * NH * HD)
    ISQ = float(1.0 / np.sqrt(HD))

    def tiled(src):
        """[(i 128), c] DRAM slice -> [128, i, c] AP (partition-major)."""
        return src.rearrange("(i p) c -> i p c", p=128).transpose([1, 0, 2])

    with tile.TileContext(nc) as tc, \
         nc.allow_low_precision(reason="bf16 ternary kernel"):
        with tc.tile_pool(name="cpool", bufs=1) as cpool, \
             tc.tile_pool(name="dbounce", bufs=1, space="DRAM") as dbounce:
            # DRAM bounce tiles for the collectives
            arq_in = dbounce.tile([1, 8], F32, name="arq_in")
            arq_out = dbounce.tile([1, 8], F32, name="arq_out",
                                   addr_space="Shared")
            aro_in = dbounce.tile([1, 8], F32, name="aro_in")
            aro_out = dbounce.tile([1, 8], F32, name="aro_out",
                                   addr_space="Shared")
            agin = [[dbounce.tile([QH * HD, 512], BF, name=f"agi{b}{qc}")
                     for qc in range(SQC)] for b in range(B)]
            agout = [[dbounce.tile([H, 512], BF, name=f"ago{b}{qc}",
                                   addr_space="Shared")
                      for qc in range(SQC)] for b in range(B)]

            # constants
            idnb = cpool.tile([128, 128], BF, name="idnb")
            nc.sync.dma_start(out=idnb[:], in_=idnb_c[:, :])
            oneskb = cpool.tile([128, 1], BF, name="oneskb")
            nc.vector.memset(oneskb[:], 1.0)
            onesk = cpool.tile([128, 1], F32, name="onesk")
            nc.sync.dma_start(out=onesk[:], in_=onesk_c[:, :])
            onesmr = cpool.tile([1, 128], F32R, name="onesmr")
            nc.sync.dma_start(out=onesmr[:], in_=onesm_c[:, :].bitcast(F32R))
            invf = cpool.tile([128, 1], F32, name="invf")
            nc.sync.dma_start(out=invf[:], in_=invf_c[:, :])
            mask_sb = cpool.tile([128, n_blk * 512], BF, name="mask_sb")
            nc.sync.dma_start(out=mask_sb[:], in_=tiled(maskP[:, :]))
            # broadcast scalars: 0 thq 1 thqn 2 thk 3 thkn 4 thv 5 thvn
            #                    6 cqk 7 gv | 8 tho 9 thon 10 go
            bsc = cpool.tile([128, 12], F32, name="bsc")
            gw = cpool.tile([128, 4], F32, name="gw")
            arq128 = cpool.tile([128, 8], F32, name="arq128")
            aro128 = cpool.tile([128, 8], F32, name="aro128")

            with tc.tile_pool(name="tab", bufs=1) as tab:
                cos_sb = tab.tile([128, T], F32, name="cos_sb")
                ss_sb = tab.tile([128, T], F32, name="ss_sb")
                # RoPE tables: Cody-Waite range reduction + Sin (emitted
                # first so they run during the initial weight DMA).
                with tc.tile_pool(name="rtab", bufs=3) as rtab:
                    for tcn in range(NTC):
                        cs = slice(tcn * 512, (tcn + 1) * 512)
                        pf = rtab.tile([128, 512], F32, name=f"pf{tcn}",
                                       tag="pf")
                        nc.scalar.dma_start(
                            out=pf[:],
                            in_=pos[0:1, cs].partition_broadcast(128))
                        f_sb = rtab.tile([128, 512], F32, name=f"f{tcn}",
                                         tag="f")
                        nc.scalar.activation(f_sb[:], pf[:], ACTF.Copy,
                                             scale=invf[:])
                        k_sb = rtab.tile([128, 512], F32, name=f"kk{tcn}",
                                         tag="kk")
                        nc.vector.tensor_scalar(k_sb[:], f_sb[:],
                                                1.0 / TWO_PI, C_MAGIC,
                                                ALU.mult, ALU.add)
                        nc.vector.tensor_scalar(k_sb[:], k_sb[:], C_MAGIC,
                                                None, ALU.subtract)
                        y_sb = rtab.tile([128, 512], F32, name=f"y{tcn}",
                                         tag="y")
                        nc.vector.scalar_tensor_tensor(
                            y_sb[:], k_sb[:], -c1, f_sb[:], ALU.mult,
                            ALU.add)
                        nc.vector.scalar_tensor_tensor(
                            y_sb[:], k_sb[:], -c2, y_sb[:], ALU.mult,
                            ALU.add)
                        nc.vector.scalar_tensor_tensor(
                            y_sb[:], k_sb[:], -c3, y_sb[:], ALU.mult,
                            ALU.add)
                        nc.scalar.activation(ss_sb[0:64, cs], y_sb[0:64, :],
                                             ACTF.Sin, scale=-1.0)
                        nc.scalar.activation(ss_sb[64:128, cs],
                                             y_sb[64:128, :], ACTF.Sin)
                        yc = rtab.tile([128, 512], F32, name=f"yc{tcn}",
                                       tag="yc")
                        nc.vector.tensor_scalar(yc[:], y_sb[:],
                                                float(np.pi / 2), None,
                                                ALU.add)
                        m_sb = rtab.tile([128, 512], F32, name=f"mm{tcn}",
                                         tag="mm")
                        nc.vector.tensor_scalar(m_sb[:], yc[:],
                                                float(np.pi), None,
                                                ALU.is_gt)
                        nc.vector.scalar_tensor_tensor(
                            yc[:], m_sb[:], -TWO_PI, yc[:], ALU.mult,
                            ALU.add)
                        nc.scalar.activation(cos_sb[:, cs], yc[:], ACTF.Sin)

                # ---- gamma prepass FIRST, with nearly all of SBUF
                # available for deep slab pipelines (result pools are
                # allocated only after these close) ----
                gacc = tc.alloc_tile_pool(name="gacc", bufs=1)
                accq = gacc.tile([128, NKT], F32, name="accq")
                acck = gacc.tile([128, NKT], F32, name="acck")
                accv = gacc.tile([128, NKT], F32, name="accv")
                acco = gacc.tile([128, NKT], F32, name="acco")
                g4 = gacc.tile([128, 4], F32, name="g4")

                with tc.tile_pool(name="wqpre", bufs=4) as wqpre, \
                     tc.tile_pool(name="wopre", bufs=3) as wopre, \
                     tc.tile_pool(name="wkvp", bufs=3) as wkvp:
                    for j in range(4):
                        sl = wqpre.tile([128, 8 * 512], F32, name=f"wqp{j}",
                                        tag="wqp")
                        nc.sync.dma_start(
                            out=sl[:],
                            in_=tiled(wqT[j * 1024:(j + 1) * 1024, :]))
                        for i in range(8):
                            nc.vector.tensor_reduce(
                                accq[:, j * 8 + i:j * 8 + i + 1],
                                sl[:, i * 512:(i + 1) * 512], X, ALU.add,
                                apply_absolute_value=True)
                    for j in range(2):
                        sl = wkvp.tile([128, 16 * 128], F32, name=f"wkp{j}",
                                       tag="wkv")
                        nc.sync.dma_start(
                            out=sl[:],
                            in_=tiled(wkT[j * 2048:(j + 1) * 2048, :]))
                        for i in range(16):
                            nc.vector.tensor_reduce(
                                acck[:, j * 16 + i:j * 16 + i + 1],
                                sl[:, i * 128:(i + 1) * 128], X, ALU.add,
                                apply_absolute_value=True)
                    for j in range(2):
                        sl = wkvp.tile([128, 16 * 128], F32, name=f"wvp{j}",
                                       tag="wkv")
                        nc.sync.dma_start(
                            out=sl[:],
                            in_=tiled(wvT[j * 2048:(j + 1) * 2048, :]))
                        for i in range(16):
                            nc.vector.tensor_reduce(
                                accv[:, j * 16 + i:j * 16 + i + 1],
                                sl[:, i * 128:(i + 1) * 128], X, ALU.add,
                                apply_absolute_value=True)
                    nc.vector.tensor_reduce(g4[:, 0:1], accq[:], X, ALU.add)
                    nc.vector.tensor_reduce(g4[:, 1:2], acck[:], X, ALU.add)
                    nc.vector.tensor_reduce(g4[:, 2:3], accv[:], X, ALU.add)
                    # Wo |.| sums ride along behind the q/k/v ones
                    for j in range(4):
                        sl = wopre.tile([128, 8 * 512], F32, name=f"wop{j}",
                                        tag="wop")
                        nc.gpsimd.dma_start(
                            out=sl[:],
                            in_=tiled(woT[j * 1024:(j + 1) * 1024, :]))
                        for i in range(8):
                            nc.vector.tensor_reduce(
                                acco[:, j * 8 + i:j * 8 + i + 1],
                                sl[:, i * 512:(i + 1) * 512], X, ALU.add,
                                apply_absolute_value=True)
                    nc.vector.tensor_reduce(g4[:, 3:4], acco[:], X, ALU.add)

                with tc.tile_pool(name="pgam", bufs=1, space="PSUM") \
                        as pgam:
                    # AllReduce #1: q/k/v gamma sums
                    pg_q = pgam.tile([1, 3], F32, name="pg_q", tag="pg")
                    nc.tensor.matmul(pg_q[:], onesk[:], g4[:, 0:3],
                                     start=True, stop=True)
                    gq_sb = gacc.tile([1, 8], F32, name="gq_sb")
                    nc.vector.memset(gq_sb[:], 0.0)
                    nc.scalar.copy(gq_sb[:, 0:3], pg_q[:])
                    nc.sync.dma_start(out=arq_in[:], in_=gq_sb[:])
                    nc.gpsimd.collective_compute(
                        "AllReduce", ALU.add, replica_groups=RG,
                        ins=[arq_in[:].opt()], outs=[arq_out[:].opt()])

                    # AllReduce #2: Wo gamma sum (issued right behind #1;
                    # result consumed at the start of phase 2)
                    pg_o = pgam.tile([1, 1], F32, name="pg_o", tag="pg")
                    nc.tensor.matmul(pg_o[:], onesk[:], g4[:, 3:4],
                                     start=True, stop=True)
                    go_sb = gacc.tile([1, 8], F32, name="go_sb")
                    nc.vector.memset(go_sb[:], 0.0)
                    nc.scalar.copy(go_sb[:, 0:1], pg_o[:])
                    nc.scalar.dma_start(out=aro_in[:], in_=go_sb[:])
                    nc.gpsimd.collective_compute(
                        "AllReduce", ALU.add, replica_groups=RG,
                        ins=[aro_in[:].opt()], outs=[aro_out[:].opt()])

                # partition-broadcast readback: thresholds become pure
                # Vector work, no PE/Scalar hops before quant can start
                nc.scalar.dma_start(
                    out=arq128[:],
                    in_=arq_out[:, :].partition_broadcast(128))
                nc.vector.tensor_scalar(gw[:, 0:1], arq128[:, 0:1],
                                        1.0 / NQ, 1e-5, ALU.mult, ALU.add)
                nc.vector.tensor_scalar(gw[:, 1:3], arq128[:, 1:3],
                                        1.0 / NK, 1e-5, ALU.mult, ALU.add)
                nc.vector.tensor_scalar(bsc[:, 0:1], gw[:, 0:1], 0.5,
                                        None, ALU.mult)
                nc.vector.tensor_scalar(bsc[:, 1:2], gw[:, 0:1], -0.5,
                                        None, ALU.mult)
                nc.vector.tensor_scalar(bsc[:, 2:3], gw[:, 1:2], 0.5,
                                        None, ALU.mult)
                nc.vector.tensor_scalar(bsc[:, 3:4], gw[:, 1:2], -0.5,
                                        None, ALU.mult)
                nc.vector.tensor_scalar(bsc[:, 4:5], gw[:, 2:3], 0.5,
                                        None, ALU.mult)
                nc.vector.tensor_scalar(bsc[:, 5:6], gw[:, 2:3], -0.5,
                                        None, ALU.mult)
                nc.vector.tensor_mul(bsc[:, 6:7], gw[:, 0:1], gw[:, 1:2])
                nc.vector.tensor_scalar(bsc[:, 6:7], bsc[:, 6:7], ISQ,
                                        None, ALU.mult)
                nc.vector.tensor_copy(bsc[:, 7:8], gw[:, 2:3])

                THQ, THQN = bsc[:, 0:1], bsc[:, 1:2]
                THK, THKN = bsc[:, 2:3], bsc[:, 3:4]
                THV, THVN = bsc[:, 4:5], bsc[:, 5:6]
                CQK, GV = bsc[:, 6:7], bsc[:, 7:8]
                THO, THON, GO = bsc[:, 8:9], bsc[:, 9:10], bsc[:, 10:11]

                def quant_tile(pool, src, dst, thp, thn, tg):
                    scr = pool.tile([128, src.shape[1]], F32,
                                    name=f"qs_{tg}", tag=f"qs{tg[0]}")
                    nc.vector.tensor_scalar(scr[:], src, thn, None,
                                            ALU.is_lt)
                    nc.vector.scalar_tensor_tensor(
                        dst, src, thp, scr[:], ALU.is_gt, ALU.subtract)

                # ---- result pools (allocated now that the prepass slabs
                # are gone), ordered by lifetime for LIFO release ----
                w3 = tc.alloc_tile_pool(name="w3", bufs=1)
                wo_sb = w3.tile([128, NKT * MSH], BF, name="wo_sb")
                qkvp = tc.alloc_tile_pool(name="qkv", bufs=1)
                qT_sb = [qkvp.tile([128, T], BF, name=f"qT{h}")
                         for h in range(QH)]
                kT_sb = qkvp.tile([128, T], BF, name="kT_sb")
                vT_sb = qkvp.tile([128, T], BF, name="vT_sb")
                vnatp = tc.alloc_tile_pool(name="vnatp", bufs=2)
                vnats = []
                wop2 = tc.alloc_tile_pool(name="wop2", bufs=2)
                qsc2 = tc.alloc_tile_pool(name="qsc2", bufs=2)
                wbig = tc.alloc_tile_pool(name="wbig", bufs=1)
                wq_sb = wbig.tile([128, NKT * 512], BF, name="wq_sb")
                wbig2 = tc.alloc_tile_pool(name="wbig2", bufs=1)
                wk_sb = wbig2.tile([128, NKT * 128], BF, name="wk_sb")
                wv_sb = wbig2.tile([128, NKT * 128], BF, name="wv_sb")

                # ---- quant q/k/v -> exact ternary bf16 (weights re-read
                # in slabs that prefetch during the AllReduce wait)
                with tc.tile_pool(name="wqst", bufs=2) as wqst, \
                     tc.tile_pool(name="wkst", bufs=4) as wkst, \
                     tc.tile_pool(name="qscr", bufs=2) as qscr:
                    wkv2 = []
                    for nm, t in (("k2", wkT), ("v2", wvT)):
                        for j in range(2):
                            sl = wkst.tile([128, 16 * 128], F32,
                                           name=f"w{nm}_{j}", tag="wkv2")
                            nc.sync.dma_start(
                                out=sl[:],
                                in_=tiled(t[j * 2048:(j + 1) * 2048, :]))
                            wkv2.append(sl)
                    for j in range(8):
                        sl = wqst.tile([128, 4 * 512], F32,
                                       name=f"wq2_{j}", tag="wq2")
                        nc.sync.dma_start(
                            out=sl[:],
                            in_=tiled(wqT[j * 512:(j + 1) * 512, :]))
                        for i in range(4):
                            k = j * 4 + i
                            quant_tile(qscr, sl[:, i * 512:(i + 1) * 512],
                                       wq_sb[:, k * 512:(k + 1) * 512],
                                       THQ, THQN, f"q{k}")
                            quant_tile(qscr,
                                       wkv2[k // 16][:,
                                                     (k % 16) * 128:
                                                     (k % 16 + 1) * 128],
                                       wk_sb[:, k * 128:(k + 1) * 128],
                                       THK, THKN, f"k{k}")
                            quant_tile(qscr,
                                       wkv2[2 + k // 16][:,
                                                         (k % 16) * 128:
                                                         (k % 16 + 1)
                                                         * 128],
                                       wv_sb[:, k * 128:(k + 1) * 128],
                                       THV, THVN, f"v{k}")

                # ---- phase 1: QKV projections + RoPE + Wo quant ----
                with tc.tile_pool(name="xin", bufs=4) as xin, \
                     tc.tile_pool(name="rope", bufs=2) as rope, \
                     tc.tile_pool(name="p1", bufs=8, space="PSUM") as p1:
                    for tcn in range(NTC):
                        cs = slice(tcn * 512, (tcn + 1) * 512)
                        xsl = []
                        for j in range(4):
                            sl = xin.tile([128, 8 * 512], BF,
                                          name=f"x{tcn}_{j}", tag="xt")
                            nc.sync.dma_start(
                                out=sl[:],
                                in_=tiled(xT[j * 1024:(j + 1) * 1024, cs]))
                            xsl.append(sl)
                        pq = [p1.tile([128, 512], F32, name=f"pq{tcn}_{h}",
                                      tag="p1") for h in range(QH)]
                        pk = p1.tile([128, 512], F32, name=f"pk{tcn}",
                                     tag="p1")
                        pv = p1.tile([128, 512], F32, name=f"pv{tcn}",
                                     tag="p1")
                        for kt in range(NKT):
                            xt_ = xsl[kt // 8][:, (kt % 8) * 512:
                                               (kt % 8 + 1) * 512]
                            st, sp = (kt == 0), (kt == NKT - 1)
                            for h in range(QH):
                                nc.tensor.matmul(
                                    pq[h][:],
                                    wq_sb[:, kt * 512 + h * 128:
                                          kt * 512 + (h + 1) * 128],
                                    xt_, start=st, stop=sp,
                                    skip_group_check=True)
                            nc.tensor.matmul(
                                pk[:], wk_sb[:, kt * 128:(kt + 1) * 128],
                                xt_, start=st, stop=sp,
                                skip_group_check=True)
                            nc.tensor.matmul(
                                pv[:], wv_sb[:, kt * 128:(kt + 1) * 128],
                                xt_, start=st, stop=sp,
                                skip_group_check=True)

                        def rope_apply(psrc, dst_ap, tg):
                            m1 = rope.tile([128, 512], F32, name=f"m1{tg}",
                                           tag="m1")
                            nc.vector.tensor_mul(m1[:], psrc[:],
                                                 cos_sb[:, cs])
                            m2 = rope.tile([128, 512], F32, name=f"m2{tg}",
                                           tag="m2")
                            nc.vector.tensor_mul(m2[0:64, :],
                                                 psrc[64:128, :],
                                                 ss_sb[0:64, cs])
                            nc.vector.tensor_mul(m2[64:128, :],
                                                 psrc[0:64, :],
                                                 ss_sb[64:128, cs])
                            nc.vector.tensor_add(dst_ap, m1[:], m2[:])

                        for h in range(QH):
                            rope_apply(pq[h], qT_sb[h][:, cs], f"_{tcn}_{h}")
                        rope_apply(pk, kT_sb[:, cs], f"k_{tcn}")
                        nc.scalar.activation(vT_sb[:, cs], pv[:], ACTF.Copy,
                                             scale=GV)

                        # vnat transposes ride along once their vT
                        # chunks exist (b0 after tcn1, b1 after tcn3)
                        if tcn % 2 == 1:
                            b = tcn // 2
                            vnat = vnatp.tile([128, S], BF,
                                              name=f"vnat{b}", tag="vnat")
                            vnats.append(vnat)
                            for kt in range(SKT):
                                ptr = p1.tile([128, 512], BF,
                                              name=f"ptr{b}_{kt}",
                                              tag="p1")
                                nc.tensor.transpose(
                                    ptr[:, 0:128],
                                    vT_sb[:, b * S + kt * 128:
                                          b * S + (kt + 1) * 128],
                                    idnb[:])
                                nc.vector.tensor_copy(
                                    vnat[:, kt * 128:(kt + 1) * 128],
                                    ptr[:, 0:128])

                wbig2.release()
                wbig.release()

                # ---- phase 2: attention (all-bf16 PE path) ----
                with tc.tile_pool(name="epool", bufs=8) as epool, \
                     tc.tile_pool(name="aop", bufs=4) as aop, \
                     tc.tile_pool(name="zpool", bufs=2) as zpool, \
                     tc.tile_pool(name="ps_s", bufs=4,
                                  space="PSUM") as ps_s, \
                     tc.tile_pool(name="ps_o", bufs=2,
                                  space="PSUM") as ps_o, \
                     tc.tile_pool(name="ps_x", bufs=1,
                                  space="PSUM") as ps_x:
                    # Wo thresholds (AR2 finished long ago) + slab DMAs;
                    # the quant compares interleave with attention below
                    nc.scalar.dma_start(
                        out=aro128[:],
                        in_=aro_out[:, :].partition_broadcast(128))
                    nc.vector.tensor_scalar(gw[:, 3:4], aro128[:, 0:1],
                                            1.0 / NO, 1e-5, ALU.mult,
                                            ALU.add)
                    nc.vector.tensor_scalar(bsc[:, 8:9], gw[:, 3:4], 0.5,
                                            None, ALU.mult)
                    nc.vector.tensor_scalar(bsc[:, 9:10], gw[:, 3:4], -0.5,
                                            None, ALU.mult)
                    nc.vector.tensor_copy(bsc[:, 10:11], gw[:, 3:4])
                    wosl = []

                    def wo_slab_fetch():
                        j = len(wosl)
                        if j >= 8:
                            return
                        wt = wop2.tile([128, 4 * 512], F32,
                                       name=f"wo2_{j}", tag="wo2")
                        nc.gpsimd.dma_start(
                            out=wt[:],
                            in_=tiled(woT[j * 512:(j + 1) * 512, :]))
                        wosl.append(wt)

                    wo_slab_fetch()
                    wo_slab_fetch()
                    woq_state = [0]

                    def wo_quant_some(n):
                        for _ in range(n):
                            k = woq_state[0]
                            if k >= NKT:
                                return
                            woq_state[0] += 1
                            if k % 4 == 0:
                                wo_slab_fetch()
                            quant_tile(
                                qsc2,
                                wosl[k // 4][:, (k % 4) * 512:
                                             (k % 4 + 1) * 512],
                                wo_sb[:, k * MSH:(k + 1) * MSH],
                                THO, THON, f"o{k}")

                    for b in range(B):
                        boff = b * S
                        vnat = vnats[b]
                        for qc in range(SQC):
                            kts = [kt for kt in range(SKT)
                                   if status[b, kt, qc] != 1]
                            assert kts, "fully-masked softmax row"
                            for h in range(QH):
                                qsl = qT_sb[h][:, boff + qc * 512:
                                               boff + (qc + 1) * 512]
                                # scores stream on the PE; the causal mask
                                # is added by an accumulating idn @ mask
                                # matmul (no Vector hop in the chain)
                                es = []
                                for kt in kts:
                                    masked = status[b, kt, qc] == 2
                                    ps_ = ps_s.tile([128, 512], F32,
                                                    name=f"s{b}{h}{qc}{kt}",
                                                    tag="ps")
                                    nc.tensor.matmul(
                                        ps_[:],
                                        kT_sb[:, boff + kt * 128:
                                              boff + (kt + 1) * 128],
                                        qsl, start=True, stop=not masked,
                                        skip_group_check=True)
                                    if masked:
                                        mi = blk_idx[(b, kt, qc)]
                                        nc.tensor.matmul(
                                            ps_[:], idnb[:],
                                            mask_sb[:, mi * 512:
                                                    (mi + 1) * 512],
                                            start=False, stop=True,
                                            skip_group_check=True)
                                    e = epool.tile([128, 512], BF,
                                                   name=f"e{b}{h}{qc}{kt}",
                                                   tag="e")
                                    nc.scalar.activation(e[:], ps_[:],
                                                         ACTF.Exp,
                                                         scale=CQK)
                                    es.append(e)
                                pz = ps_x.tile([1, 512], F32,
                                               name=f"pz{b}{h}{qc}",
                                               tag="pz")
                                po = ps_o.tile([128, 512], F32,
                                               name=f"po{b}{h}{qc}",
                                               tag="po")
                                for i, kt in enumerate(kts):
                                    fst = (i == 0)
                                    lst = (i == len(kts) - 1)
                                    nc.tensor.matmul(
                                        pz[:], oneskb[:], es[i][:],
                                        start=fst, stop=lst,
                                        skip_group_check=True)
                                    nc.tensor.matmul(
                                        po[:],
                                        vnat[:, kt * 128:(kt + 1) * 128],
                                        es[i][:], start=fst, stop=lst,
                                        skip_group_check=True)
                                # z -> SBUF, ones-broadcast, full-width
                                # reciprocal (a [1,512] recip is 1-lane)
                                zsb = zpool.tile([1, 512], F32R,
                                                 name=f"zs{b}{h}{qc}",
                                                 tag="zs")
                                nc.scalar.copy(zsb[:], pz[:])
                                pzb = ps_x.tile([128, 512], F32,
                                                name=f"pzb{b}{h}{qc}",
                                                tag="pzb")
                                nc.tensor.matmul(pzb[:], onesmr[:], zsb[:],
                                                 start=True, stop=True,
                                                 skip_group_check=True)
                                zb = zpool.tile([128, 512], F32,
                                                name=f"zb{b}{h}{qc}",
                                                tag="zb")
                                nc.vector.reciprocal(zb[:], pzb[:])
                                ao = aop.tile([128, 512], BF,
                                              name=f"ao{b}{h}{qc}",
                                              tag="ao")
                                nc.vector.tensor_mul(ao[:], po[:], zb[:])
                                nc.sync.dma_start(
                                    out=agin[b][qc][h * 128:(h + 1) * 128,
                                                    :],
                                    in_=ao[:])
                                wo_quant_some(3)
                            nc.gpsimd.collective_compute(
                                "AllGather", ALU.bypass, replica_groups=RG,
                                ins=[agin[b][qc][:].opt()],
                                outs=[agout[b][qc][:].opt()])

                qsc2.release()
                wop2.release()

                # ---- phase 3: o_proj, per (batch, qchunk) for overlap ----
                with tc.tile_pool(name="a3", bufs=3) as a3, \
                     tc.tile_pool(name="o3", bufs=2) as o3, \
                     tc.tile_pool(name="p3", bufs=4, space="PSUM") as p3:
                    for ch in range(NTC):
                        b, q2 = ch // 2, ch % 2
                        at = a3.tile([128, NKT * 512], BF, name=f"at{ch}",
                                     tag="at")
                        nc.sync.dma_start(out=at[:],
                                          in_=tiled(agout[b][q2][:, :]))
                        for tt in range(4):
                            pout = p3.tile([128, 512], F32,
                                           name=f"po3_{ch}{tt}", tag="pout")
                            for kt in range(NKT):
                                nc.tensor.matmul(
                                    pout[:],
                                    at[:, kt * 512 + tt * 128:
                                       kt * 512 + (tt + 1) * 128],
                                    wo_sb[:, kt * MSH:(kt + 1) * MSH],
                                    start=(kt == 0), stop=(kt == NKT - 1),
                                    skip_group_check=True)
                            osb = o3.tile([128, 512], F32,
                                          name=f"osb{ch}{tt}", tag="osb")
                            nc.scalar.activation(osb[:], pout[:], ACTF.Copy,
                                                 scale=GO)
                            nc.sync.dma_start(
                                out=outN[ch * 512 + tt * 128:
                                         ch * 512 + (tt + 1) * 128, :],
                                in_=osb[:])
                vnatp.release()
                qkvp.release()
                w3.release()
                gacc.release()

    nc.compile()
    return nc


def kernel(hidden_states, Wq, Wk, Wv, Wo, attention_mask, position_ids):
    from concourse.bass_utils import run_bass_kernel_spmd
    from concourse.bass_interp import get_hw_module

    hs = np.ascontiguousarray(np.asarray(hidden_states, dtype=np.float32))
    Wq = np.asarray(Wq, dtype=np.float32)
    Wk = np.asarray(Wk, dtype=np.float32)
    Wv = np.asarray(Wv, dtype=np.float32)
    Wo = np.asarray(Wo, dtype=np.float32)
    mask = np.asarray(attention_mask, dtype=np.float32)
    posf = np.ascontiguousarray(
        np.asarray(position_ids).reshape(1, T).astype(np.float32))

    status, blk_idx, packed = _classify_mask(mask)
    n_blk = packed.shape[0] // 128
    assert n_blk <= 16, "too many distinct mask blocks"

    key = (status.tobytes(), tuple(sorted(blk_idx.items())), n_blk)
    if key not in _cache:
        nc = _build(status, blk_idx, n_blk)
        nc.m = get_hw_module(nc.m)
        _cache[key] = nc
    nc = _cache[key]

    xT = np.ascontiguousarray(hs.reshape(T, H).T.astype(BF16NP))
    in_maps = []
    for c in range(NCORES):
        in_maps.append({
            "xT": xT,
            "wqT": np.ascontiguousarray(
                Wq[c * QH * HD:(c + 1) * QH * HD, :].T),
            "wkT": np.ascontiguousarray(Wk[c * HD:(c + 1) * HD, :].T),
            "wvT": np.ascontiguousarray(Wv[c * HD:(c + 1) * HD, :].T),
            "woT": np.ascontiguousarray(Wo[c * MSH:(c + 1) * MSH, :].T),
            "maskP": packed,
            "pos": posf,
        })
    res = run_bass_kernel_spmd(nc, in_maps, core_ids=list(range(NCORES)),
                               trace=bool(os.environ.get("BITNET_TRACE")))
    global last_exec_time_ns
    last_exec_time_ns = res.exec_time_ns
    out = np.concatenate(
        [res.results[c]["outN"] for c in range(NCORES)], axis=1)  # (T, MSH*8)
    return np.ascontiguousarray(out).reshape(B, S, H).astype(np.float32)


# revision 26
# speedup vs baseline: 1.0714x; 1.0423x over previous
"""BitNet attention block on 8 TRN2 NeuronCores (tensor-parallel over heads).

Self-contained: kernel(**inputs) takes full inputs, shards internally,
runs one SPMD Bass program on cores 0-7, reassembles the full output.

Sharding: core c owns Q heads [4c,4c+4), KV head c, o_proj output dims
[512c, 512c+512). Attention is fully local per core. Cross-core comms:
two tiny AllReduces for the BitNet absmean gammas and four bf16
AllGathers (one per (batch, 512-token chunk)) of the attention output.

Key optimizations vs the fp32r baseline:
- Exact ternary {-1,0,+1} weights stored bf16; gamma scales folded into
  activation `scale=` APs (exp carries gq*gk/sqrt(HD), the V copy gv,
  the o_proj copy go). x is bf16 (host-converted). Quant decisions
  reproduce round-half-even+clip via wq = (w > g/2) - (w < -g/2).
- Everything the PE streams in the hot loops is bf16 (fp32r moving
  operands measure ~2x slower on HW than the cost model claims).
- The causal mask is added on the PE itself (an accumulating
  identity @ mask matmul into the score PSUM group) so the
  score->exp chain never crosses through the Vector engine.
- Softmax normalization: ones-broadcast matmul of the PSUM z-row,
  then a full-width [128,512] reciprocal (a [1,512] reciprocal is
  single-lane and costs 3.3us).
- Gamma AllReduce results are read back with a partition-broadcast
  DMA so thresholds are pure Vector work (no PE/Scalar hops on the
  critical path). AR2-dependent Wo work is emitted mid-phase-1.
- DMA descriptor issue costs ~0.6us, so all tile loads ride
  multi-tile slab DMAs built with AP rearrange+transpose.
- Phase 3 for batch b overlaps the other batch's AllGathers; the
  per-qc AllGather split lets agout land earlier.
"""
import os
import sys
sys.path.insert(0, "/opt/trn_rl_repo")
import numpy as np
import ml_dtypes

B, S, H = 2, 1024, 4096
NH, NKV, HD = 32, 8, 128
NCORES = 8
T = B * S
QH = NH // NCORES          # 4 q-heads per core
MSH = H // NCORES          # 512 o_proj out-dims per core
THETA = 10000.0
C_MAGIC = 12582912.0       # 1.5 * 2**23
TWO_PI = 6.283185307179586
NKT = H // 128             # 32 contraction tiles
NTC = T // 512             # 4 token chunks
SKT = S // 128             # 8 score k-tiles per batch
SQC = S // 512             # 2 q-chunks per batch
BF16NP = ml_dtypes.bfloat16

_cache = {}
last_exec_time_ns = None


def _classify_mask(mask):
    """Per (b, kt, qc) [128k x 512q] block: 0 no-op, 1 fully masked
    (skipped), 2 needs a mask add (index into deduped distinct blocks)."""
    status = np.empty((B, SKT, SQC), dtype=np.int8)
    blk_idx = {}
    distinct = []
    seen = {}
    for b in range(B):
        mb = np.asarray(mask[b, 0], dtype=np.float32)
        for kt in range(SKT):
            for qc in range(SQC):
                blk = mb[qc * 512:(qc + 1) * 512, kt * 128:(kt + 1) * 128]
                if not blk.any():
                    status[b, kt, qc] = 0
                elif (blk <= -1e4).all():
                    status[b, kt, qc] = 1
                else:
                    status[b, kt, qc] = 2
                    kb = blk.tobytes()
                    if kb not in seen:
                        seen[kb] = len(distinct)
                        distinct.append(np.ascontiguousarray(blk.T))
                    blk_idx[(b, kt, qc)] = seen[kb]
    if distinct:
        packed = np.concatenate(distinct, axis=0)
    else:
        packed = np.zeros((128, 512), dtype=np.float32)
    return status, blk_idx, np.ascontiguousarray(packed.astype(BF16NP))


def _cody_consts():
    c1 = float(np.float32(6.28125))
    r = np.float64(TWO_PI) - c1
    c2 = float(np.float32(r - np.remainder(r, 2.0 ** -24)))
    c3 = float(np.float32(np.float64(TWO_PI) - c1 - float(c2)))
    return c1, c2, c3


def _build(status, blk_idx, n_blk):
    from concourse import bacc, tile, mybir

    F32 = mybir.dt.float32
    F32R = mybir.dt.float32r
    BF = mybir.dt.bfloat16
    ACTF = mybir.ActivationFunctionType
    ALU = mybir.AluOpType
    X = mybir.AxisListType.X
    RG = [list(range(NCORES))]
    c1, c2, c3 = _cody_consts()

    nc = bacc.Bacc("TRN2", target_bir_lowering=False, debug=False,
                   num_devices=NCORES)

    xT = nc.dram_tensor("xT", [H, T], BF, kind="ExternalInput")
    wqT = nc.dram_tensor("wqT", [H, QH * HD], F32, kind="ExternalInput")
    wkT = nc.dram_tensor("wkT", [H, HD], F32, kind="ExternalInput")
    wvT = nc.dram_tensor("wvT", [H, HD], F32, kind="ExternalInput")
    woT = nc.dram_tensor("woT", [H, MSH], F32, kind="ExternalInput")
    maskP = nc.dram_tensor("maskP", [n_blk * 128, 512], BF,
                           kind="ExternalInput")
    pos = nc.dram_tensor("pos", [1, T], F32, kind="ExternalInput")
    outN = nc.dram_tensor("outN", [T, MSH], F32, kind="ExternalOutput")

    idnb_c = nc.inline_tensor(np.eye(128, dtype=BF16NP), name="idnb_c")
    onesm_c = nc.inline_tensor(np.ones((1, 128), np.float32), name="onesm_c")
    onesk_c = nc.inline_tensor(np.ones((128, 1), np.float32), name="onesk_c")
    invf_np = (1.0 / THETA ** (np.arange(0, HD, 2, dtype=np.float32) / HD))
    invf_np = np.concatenate([invf_np, invf_np]).reshape(HD, 1)
    invf_c = nc.inline_tensor(invf_np.astype(np.float32), name="invf_c")

    NQ = float(NH * HD * H)
    NK = float(NKV * HD * H)
    NO = float(H * NH * HD)
    ISQ = float(1.0 / np.sqrt(HD))

    def tiled(src):
        """[(i 128), c] DRAM slice -> [128, i, c] AP (partition-major)."""
        return src.rearrange("(i p) c -> i p c", p=128).transpose([1, 0, 2])

    with tile.TileContext(nc) as tc, \
         nc.allow_low_precision(reason="bf16 ternary kernel"):
        with tc.tile_pool(name="cpool", bufs=1) as cpool, \
             tc.tile_pool(name="dbounce", bufs=1, space="DRAM") as dbounce:
            # DRAM bounce tiles for the collectives
            arq_in = dbounce.tile([1, 8], F32, name="arq_in")
            arq_out = dbounce.tile([1, 8], F32, name="arq_out",
                                   addr_space="Shared")
            aro_in = dbounce.tile([1, 8], F32, name="aro_in")
            aro_out = dbounce.tile([1, 8], F32, name="aro_out",
                                   addr_space="Shared")
            agin = [[dbounce.tile([QH * HD, 512], BF, name=f"agi{b}{qc}")
                     for qc in range(SQC)] for b in range(B)]
            agout = [[dbounce.tile([H, 512], BF, name=f"ago{b}{qc}",
                                   addr_space="Shared")
                      for qc in range(SQC)] for b in range(B)]

            # constants
            idnb = cpool.tile([128, 128], BF, name="idnb")
            nc.sync.dma_start(out=idnb[:], in_=idnb_c[:, :])
            oneskb = cpool.tile([128, 1], BF, name="oneskb")
            nc.vector.memset(oneskb[:], 1.0)
            onesk = cpool.tile([128, 1], F32, name="onesk")
            nc.sync.dma_start(out=onesk[:], in_=onesk_c[:, :])
            onesmr = cpool.tile([1, 128], F32R, name="onesmr")
            nc.sync.dma_start(out=onesmr[:], in_=onesm_c[:, :].bitcast(F32R))
            invf = cpool.tile([128, 1], F32, name="invf")
            nc.sync.dma_start(out=invf[:], in_=invf_c[:, :])
            mask_sb = cpool.tile([128, n_blk * 512], BF, name="mask_sb")
            nc.sync.dma_start(out=mask_sb[:], in_=tiled(maskP[:, :]))
            # broadcast scalars: 0 thq 1 thqn 2 thk 3 thkn 4 thv 5 thvn
            #                    6 cqk 7 gv | 8 tho 9 thon 10 go
            bsc = cpool.tile([128, 12], F32, name="bsc")
            gw = cpool.tile([128, 4], F32, name="gw")
            arq128 = cpool.tile([128, 8], F32, name="arq128")
            aro128 = cpool.tile([128, 8], F32, name="aro128")

            with tc.tile_pool(name="tab", bufs=1) as tab:
                cos_sb = tab.tile([128, T], F32, name="cos_sb")
                ss_sb = tab.tile([128, T], F32, name="ss_sb")
                # RoPE tables: Cody-Waite range reduction + Sin (emitted
                # first so they run during the initial weight DMA).
                with tc.tile_pool(name="rtab", bufs=3) as rtab:
                    for tcn in range(NTC):
                        cs = slice(tcn * 512, (tcn + 1) * 512)
                        pf = rtab.tile([128, 512], F32, name=f"pf{tcn}",
                                       tag="pf")
                        nc.scalar.dma_start(
                            out=pf[:],
                            in_=pos[0:1, cs].partition_broadcast(128))
                        f_sb = rtab.tile([128, 512], F32, name=f"f{tcn}",
                                         tag="f")
                        nc.scalar.activation(f_sb[:], pf[:], ACTF.Copy,
                                             scale=invf[:])
                        k_sb = rtab.tile([128, 512], F32, name=f"kk{tcn}",
                                         tag="kk")
                        nc.vector.tensor_scalar(k_sb[:], f_sb[:],
                                                1.0 / TWO_PI, C_MAGIC,
                                                ALU.mult, ALU.add)
                        nc.vector.tensor_scalar(k_sb[:], k_sb[:], C_MAGIC,
                                                None, ALU.subtract)
                        y_sb = rtab.tile([128, 512], F32, name=f"y{tcn}",
                                         tag="y")
                        nc.vector.scalar_tensor_tensor(
                            y_sb[:], k_sb[:], -c1, f_sb[:], ALU.mult,
                            ALU.add)
                        nc.vector.scalar_tensor_tensor(
                            y_sb[:], k_sb[:], -c2, y_sb[:], ALU.mult,
                            ALU.add)
                        nc.vector.scalar_tensor_tensor(
                            y_sb[:], k_sb[:], -c3, y_sb[:], ALU.mult,
                            ALU.add)
                        nc.scalar.activation(ss_sb[0:64, cs], y_sb[0:64, :],
                                             ACTF.Sin, scale=-1.0)
                        nc.scalar.activation(ss_sb[64:128, cs],
                                             y_sb[64:128, :], ACTF.Sin)
                        yc = rtab.tile([128, 512], F32, name=f"yc{tcn}",
                                       tag="yc")
                        nc.vector.tensor_scalar(yc[:], y_sb[:],
                                                float(np.pi / 2), None,
                                                ALU.add)
                        m_sb = rtab.tile([128, 512], F32, name=f"mm{tcn}",
                                         tag="mm")
                        nc.vector.tensor_scalar(m_sb[:], yc[:],
                                                float(np.pi), None,
                                                ALU.is_gt)
                        nc.vector.scalar_tensor_tensor(
                            yc[:], m_sb[:], -TWO_PI, yc[:], ALU.mult,
                            ALU.add)
                        nc.scalar.activation(cos_sb[:, cs], yc[:], ACTF.Sin)

                # ---- gamma prepass FIRST, with nearly all of SBUF
                # available for deep slab pipelines (result pools are
                # allocated only after these close) ----
                gacc = tc.alloc_tile_pool(name="gacc", bufs=1)
                accq = gacc.tile([128, NKT], F32, name="accq")
                acck = gacc.tile([128, NKT], F32, name="acck")
                accv = gacc.tile([128, NKT], F32, name="accv")
                acco = gacc.tile([128, NKT], F32, name="acco")
                g4 = gacc.tile([128, 4], F32, name="g4")

                with tc.tile_pool(name="wqpre", bufs=4) as wqpre, \
                     tc.tile_pool(name="wopre", bufs=3) as wopre, \
                     tc.tile_pool(name="wkvp", bufs=3) as wkvp:
                    for j in range(4):
                        sl = wqpre.tile([128, 8 * 512], F32, name=f"wqp{j}",
                                        tag="wqp")
                        nc.sync.dma_start(
                            out=sl[:],
                            in_=tiled(wqT[j * 1024:(j + 1) * 1024, :]))
                        for i in range(8):
                            nc.vector.tensor_reduce(
                                accq[:, j * 8 + i:j * 8 + i + 1],
                                sl[:, i * 512:(i + 1) * 512], X, ALU.add,
                                apply_absolute_value=True)
                    for j in range(2):
                        sl = wkvp.tile([128, 16 * 128], F32, name=f"wkp{j}",
                                       tag="wkv")
                        nc.sync.dma_start(
                            out=sl[:],
                            in_=tiled(wkT[j * 2048:(j + 1) * 2048, :]))
                        for i in range(16):
                            nc.vector.tensor_reduce(
                                acck[:, j * 16 + i:j * 16 + i + 1],
                                sl[:, i * 128:(i + 1) * 128], X, ALU.add,
                                apply_absolute_value=True)
                    for j in range(2):
                        sl = wkvp.tile([128, 16 * 128], F32, name=f"wvp{j}",
                                       tag="wkv")
                        nc.sync.dma_start(
                            out=sl[:],
                            in_=tiled(wvT[j * 2048:(j + 1) * 2048, :]))
                        for i in range(16):
                            nc.vector.tensor_reduce(
                                accv[:, j * 16 + i:j * 16 + i + 1],
                                sl[:, i * 128:(i + 1) * 128], X, ALU.add,
                                apply_absolute_value=True)
                    nc.vector.tensor_reduce(g4[:, 0:1], accq[:], X, ALU.add)
                    nc.vector.tensor_reduce(g4[:, 1:2], acck[:], X, ALU.add)
                    nc.vector.tensor_reduce(g4[:, 2:3], accv[:], X, ALU.add)
                    # Wo |.| sums ride along behind the q/k/v ones
                    for j in range(4):
                        sl = wopre.tile([128, 8 * 512], F32, name=f"wop{j}",
                                        tag="wop")
                        nc.gpsimd.dma_start(
                            out=sl[:],
                            in_=tiled(woT[j * 1024:(j + 1) * 1024, :]))
                        for i in range(8):
                            nc.vector.tensor_reduce(
                                acco[:, j * 8 + i:j * 8 + i + 1],
                                sl[:, i * 512:(i + 1) * 512], X, ALU.add,
                                apply_absolute_value=True)
                    nc.vector.tensor_reduce(g4[:, 3:4], acco[:], X, ALU.add)

                # AllReduce #1: q/k/v gamma sums.  The Vector engine
                # issues its own result readback so no other queue's
                # backlog can delay the quant start.
                with tc.tile_pool(name="pgq", bufs=1, space="PSUM") as pgq:
                    pg_q = pgq.tile([1, 3], F32, name="pg_q", tag="pg")
                    nc.tensor.matmul(pg_q[:], onesk[:], g4[:, 0:3],
                                     start=True, stop=True)
                    gq_sb = gacc.tile([1, 8], F32, name="gq_sb")
                    nc.vector.memset(gq_sb[:], 0.0)
                    nc.scalar.copy(gq_sb[:, 0:3], pg_q[:])
                    nc.sync.dma_start(out=arq_in[:], in_=gq_sb[:])
                    nc.gpsimd.collective_compute(
                        "AllReduce", ALU.add, replica_groups=RG,
                        ins=[arq_in[:].opt()], outs=[arq_out[:].opt()])
                nc.scalar.dma_start(
                    out=arq128[:],
                    in_=arq_out[:, :].partition_broadcast(128))
                nc.vector.tensor_scalar(gw[:, 0:1], arq128[:, 0:1],
                                        1.0 / NQ, 1e-5, ALU.mult, ALU.add)
                nc.vector.tensor_scalar(gw[:, 1:3], arq128[:, 1:3],
                                        1.0 / NK, 1e-5, ALU.mult, ALU.add)
                nc.vector.tensor_scalar(bsc[:, 0:1], gw[:, 0:1], 0.5,
                                        None, ALU.mult)
                nc.vector.tensor_scalar(bsc[:, 1:2], gw[:, 0:1], -0.5,
                                        None, ALU.mult)
                nc.vector.tensor_scalar(bsc[:, 2:3], gw[:, 1:2], 0.5,
                                        None, ALU.mult)
                nc.vector.tensor_scalar(bsc[:, 3:4], gw[:, 1:2], -0.5,
                                        None, ALU.mult)
                nc.vector.tensor_scalar(bsc[:, 4:5], gw[:, 2:3], 0.5,
                                        None, ALU.mult)
                nc.vector.tensor_scalar(bsc[:, 5:6], gw[:, 2:3], -0.5,
                                        None, ALU.mult)
                nc.vector.tensor_mul(bsc[:, 6:7], gw[:, 0:1], gw[:, 1:2])
                nc.vector.tensor_scalar(bsc[:, 6:7], bsc[:, 6:7], ISQ,
                                        None, ALU.mult)
                nc.vector.tensor_copy(bsc[:, 7:8], gw[:, 2:3])

                THQ, THQN = bsc[:, 0:1], bsc[:, 1:2]
                THK, THKN = bsc[:, 2:3], bsc[:, 3:4]
                THV, THVN = bsc[:, 4:5], bsc[:, 5:6]
                CQK, GV = bsc[:, 6:7], bsc[:, 7:8]
                THO, THON, GO = bsc[:, 8:9], bsc[:, 9:10], bsc[:, 10:11]

                def quant_tile(pool, src, dst, thp, thn, tg):
                    scr = pool.tile([128, src.shape[1]], F32,
                                    name=f"qs_{tg}", tag=f"qs{tg[0]}")
                    nc.vector.tensor_scalar(scr[:], src, thn, None,
                                            ALU.is_lt)
                    nc.vector.scalar_tensor_tensor(
                        dst, src, thp, scr[:], ALU.is_gt, ALU.subtract)

                # ---- result pools (allocated now that the prepass slabs
                # are gone), ordered by lifetime for LIFO release ----
                w3 = tc.alloc_tile_pool(name="w3", bufs=1)
                wo_sb = w3.tile([128, NKT * MSH], BF, name="wo_sb")
                qkvp = tc.alloc_tile_pool(name="qkv", bufs=1)
                qT_sb = [qkvp.tile([128, T], BF, name=f"qT{h}")
                         for h in range(QH)]
                kT_sb = qkvp.tile([128, T], BF, name="kT_sb")
                vT_sb = qkvp.tile([128, T], BF, name="vT_sb")
                vnatp = tc.alloc_tile_pool(name="vnatp", bufs=2)
                vnats = []
                wop2 = tc.alloc_tile_pool(name="wop2", bufs=2)
                qsc2 = tc.alloc_tile_pool(name="qsc2", bufs=2)
                wbig = tc.alloc_tile_pool(name="wbig", bufs=1)
                wq_sb = wbig.tile([128, NKT * 512], BF, name="wq_sb")
                wbig2 = tc.alloc_tile_pool(name="wbig2", bufs=1)
                wk_sb = wbig2.tile([128, NKT * 128], BF, name="wk_sb")
                wv_sb = wbig2.tile([128, NKT * 128], BF, name="wv_sb")

                # ---- quant q/k/v -> exact ternary bf16 (weights re-read
                # in slabs that prefetch during the AllReduce wait)
                with tc.tile_pool(name="wqst", bufs=2) as wqst, \
                     tc.tile_pool(name="wkst", bufs=4) as wkst, \
                     tc.tile_pool(name="qscr", bufs=2) as qscr:
                    wkv2 = []
                    for nm, t in (("k2", wkT), ("v2", wvT)):
                        for j in range(2):
                            sl = wkst.tile([128, 16 * 128], F32,
                                           name=f"w{nm}_{j}", tag="wkv2")
                            nc.sync.dma_start(
                                out=sl[:],
                                in_=tiled(t[j * 2048:(j + 1) * 2048, :]))
                            wkv2.append(sl)
                    for j in range(8):
                        sl = wqst.tile([128, 4 * 512], F32,
                                       name=f"wq2_{j}", tag="wq2")
                        nc.sync.dma_start(
                            out=sl[:],
                            in_=tiled(wqT[j * 512:(j + 1) * 512, :]))
                        for i in range(4):
                            k = j * 4 + i
                            quant_tile(qscr, sl[:, i * 512:(i + 1) * 512],
                                       wq_sb[:, k * 512:(k + 1) * 512],
                                       THQ, THQN, f"q{k}")
                            quant_tile(qscr,
                                       wkv2[k // 16][:,
                                                     (k % 16) * 128:
                                                     (k % 16 + 1) * 128],
                                       wk_sb[:, k * 128:(k + 1) * 128],
                                       THK, THKN, f"k{k}")
                            quant_tile(qscr,
                                       wkv2[2 + k // 16][:,
                                                         (k % 16) * 128:
                                                         (k % 16 + 1)
                                                         * 128],
                                       wv_sb[:, k * 128:(k + 1) * 128],
                                       THV, THVN, f"v{k}")

                # ---- phase 1: QKV projections + RoPE + Wo quant ----
                with tc.tile_pool(name="xin", bufs=4) as xin, \
                     tc.tile_pool(name="rope", bufs=2) as rope, \
                     tc.tile_pool(name="p1", bufs=8, space="PSUM") as p1:
                    for tcn in range(NTC):
                        cs = slice(tcn * 512, (tcn + 1) * 512)
                        xsl = []
                        for j in range(4):
                            sl = xin.tile([128, 8 * 512], BF,
                                          name=f"x{tcn}_{j}", tag="xt")
                            nc.sync.dma_start(
                                out=sl[:],
                                in_=tiled(xT[j * 1024:(j + 1) * 1024, cs]))
                            xsl.append(sl)
                        pq = [p1.tile([128, 512], F32, name=f"pq{tcn}_{h}",
                                      tag="p1") for h in range(QH)]
                        pk = p1.tile([128, 512], F32, name=f"pk{tcn}",
                                     tag="p1")
                        pv = p1.tile([128, 512], F32, name=f"pv{tcn}",
                                     tag="p1")
                        for kt in range(NKT):
                            xt_ = xsl[kt // 8][:, (kt % 8) * 512:
                                               (kt % 8 + 1) * 512]
                            st, sp = (kt == 0), (kt == NKT - 1)
                            for h in range(QH):
                                nc.tensor.matmul(
                                    pq[h][:],
                                    wq_sb[:, kt * 512 + h * 128:
                                          kt * 512 + (h + 1) * 128],
                                    xt_, start=st, stop=sp,
                                    skip_group_check=True)
                            nc.tensor.matmul(
                                pk[:], wk_sb[:, kt * 128:(kt + 1) * 128],
                                xt_, start=st, stop=sp,
                                skip_group_check=True)
                            nc.tensor.matmul(
                                pv[:], wv_sb[:, kt * 128:(kt + 1) * 128],
                                xt_, start=st, stop=sp,
                                skip_group_check=True)

                        def rope_apply(psrc, dst_ap, tg):
                            m1 = rope.tile([128, 512], F32, name=f"m1{tg}",
                                           tag="m1")
                            nc.vector.tensor_mul(m1[:], psrc[:],
                                                 cos_sb[:, cs])
                            m2 = rope.tile([128, 512], F32, name=f"m2{tg}",
                                           tag="m2")
                            nc.vector.tensor_mul(m2[0:64, :],
                                                 psrc[64:128, :],
                                                 ss_sb[0:64, cs])
                            nc.vector.tensor_mul(m2[64:128, :],
                                                 psrc[0:64, :],
                                                 ss_sb[64:128, cs])
                            nc.vector.tensor_add(dst_ap, m1[:], m2[:])

                        for h in range(QH):
                            rope_apply(pq[h], qT_sb[h][:, cs], f"_{tcn}_{h}")
                        rope_apply(pk, kT_sb[:, cs], f"k_{tcn}")
                        nc.scalar.activation(vT_sb[:, cs], pv[:], ACTF.Copy,
                                             scale=GV)

                        if tcn == 0:
                            # AllReduce #2 trigger (Wo gamma); the PE
                            # reaches this point well after g4[:,3]
                            pg_o = p1.tile([128, 512], F32, name="pg_o",
                                           tag="p1")
                            nc.tensor.matmul(pg_o[0:1, 0:1], onesk[:],
                                             g4[:, 3:4], start=True,
                                             stop=True)
                            go_sb = gacc.tile([1, 8], F32, name="go_sb")
                            nc.vector.memset(go_sb[:], 0.0)
                            nc.scalar.copy(go_sb[:, 0:1], pg_o[0:1, 0:1])
                            nc.scalar.dma_start(out=aro_in[:],
                                                in_=go_sb[:])
                            nc.gpsimd.collective_compute(
                                "AllReduce", ALU.add, replica_groups=RG,
                                ins=[aro_in[:].opt()],
                                outs=[aro_out[:].opt()])

                        # vnat transposes ride along once their vT
                        # chunks exist (b0 after tcn1, b1 after tcn3)
                        if tcn % 2 == 1:
                            b = tcn // 2
                            vnat = vnatp.tile([128, S], BF,
                                              name=f"vnat{b}", tag="vnat")
                            vnats.append(vnat)
                            for kt in range(SKT):
                                ptr = p1.tile([128, 512], BF,
                                              name=f"ptr{b}_{kt}",
                                              tag="p1")
                                nc.tensor.transpose(
                                    ptr[:, 0:128],
                                    vT_sb[:, b * S + kt * 128:
                                          b * S + (kt + 1) * 128],
                                    idnb[:])
                                nc.vector.tensor_copy(
                                    vnat[:, kt * 128:(kt + 1) * 128],
                                    ptr[:, 0:128])

                wbig2.release()
                wbig.release()

                # ---- phase 2: attention (all-bf16 PE path) ----
                with tc.tile_pool(name="epool", bufs=8) as epool, \
                     tc.tile_pool(name="aop", bufs=4) as aop, \
                     tc.tile_pool(name="zpool", bufs=2) as zpool, \
                     tc.tile_pool(name="ps_s", bufs=4,
                                  space="PSUM") as ps_s, \
                     tc.tile_pool(name="ps_o", bufs=2,
                                  space="PSUM") as ps_o, \
                     tc.tile_pool(name="ps_x", bufs=1,
                                  space="PSUM") as ps_x:
                    # Wo thresholds (AR2 finished long ago) + slab DMAs;
                    # the quant compares interleave with attention below
                    nc.scalar.dma_start(
                        out=aro128[:],
                        in_=aro_out[:, :].partition_broadcast(128))
                    nc.vector.tensor_scalar(gw[:, 3:4], aro128[:, 0:1],
                                            1.0 / NO, 1e-5, ALU.mult,
                                            ALU.add)
                    nc.vector.tensor_scalar(bsc[:, 8:9], gw[:, 3:4], 0.5,
                                            None, ALU.mult)
                    nc.vector.tensor_scalar(bsc[:, 9:10], gw[:, 3:4], -0.5,
                                            None, ALU.mult)
                    nc.vector.tensor_copy(bsc[:, 10:11], gw[:, 3:4])
                    wosl = []

                    def wo_slab_fetch():
                        j = len(wosl)
                        if j >= 8:
                            return
                        wt = wop2.tile([128, 4 * 512], F32,
                                       name=f"wo2_{j}", tag="wo2")
                        nc.gpsimd.dma_start(
                            out=wt[:],
                            in_=tiled(woT[j * 512:(j + 1) * 512, :]))
                        wosl.append(wt)

                    wo_slab_fetch()
                    wo_slab_fetch()
                    woq_state = [0]

                    def wo_quant_some(n):
                        for _ in range(n):
                            k = woq_state[0]
                            if k >= NKT:
                                return
                            woq_state[0] += 1
                            if k % 4 == 0:
                                wo_slab_fetch()
                            quant_tile(
                                qsc2,
                                wosl[k // 4][:, (k % 4) * 512:
                                             (k % 4 + 1) * 512],
                                wo_sb[:, k * MSH:(k + 1) * MSH],
                                THO, THON, f"o{k}")

                    for b in range(B):
                        boff = b * S
                        vnat = vnats[b]
                        for qc in range(SQC):
                            kts = [kt for kt in range(SKT)
                                   if status[b, kt, qc] != 1]
                            assert kts, "fully-masked softmax row"
                            for h in range(QH):
                                qsl = qT_sb[h][:, boff + qc * 512:
                                               boff + (qc + 1) * 512]
                                # scores stream on the PE; the causal mask
                                # is added by an accumulating idn @ mask
                                # matmul (no Vector hop in the chain)
                                es = []
                                for kt in kts:
                                    masked = status[b, kt, qc] == 2
                                    ps_ = ps_s.tile([128, 512], F32,
                                                    name=f"s{b}{h}{qc}{kt}",
                                                    tag="ps")
                                    nc.tensor.matmul(
                                        ps_[:],
                                        kT_sb[:, boff + kt * 128:
                                              boff + (kt + 1) * 128],
                                        qsl, start=True, stop=not masked,
                                        skip_group_check=True)
                                    if masked:
                                        mi = blk_idx[(b, kt, qc)]
                                        nc.tensor.matmul(
                                            ps_[:], idnb[:],
                                            mask_sb[:, mi * 512:
                                                    (mi + 1) * 512],
                                            start=False, stop=True,
                                            skip_group_check=True)
                                    e = epool.tile([128, 512], BF,
                                                   name=f"e{b}{h}{qc}{kt}",
                                                   tag="e")
                                    nc.scalar.activation(e[:], ps_[:],
                                                         ACTF.Exp,
                                                         scale=CQK)
                                    es.append(e)
                                pz = ps_x.tile([1, 512], F32,
                                               name=f"pz{b}{h}{qc}",
                                               tag="pz")
                                po = ps_o.tile([128, 512], F32,
                                               name=f"po{b}{h}{qc}",
                                               tag="po")
                                for i, kt in enumerate(kts):
                                    fst = (i == 0)
                                    lst = (i == len(kts) - 1)
                                    nc.tensor.matmul(
                                        pz[:], oneskb[:], es[i][:],
                                        start=fst, stop=lst,
                                        skip_group_check=True)
                                    nc.tensor.matmul(
                                        po[:],
                                        vnat[:, kt * 128:(kt + 1) * 128],
                                        es[i][:], start=fst, stop=lst,
                                        skip_group_check=True)
                                # z -> SBUF, ones-broadcast, full-width
                                # reciprocal (a [1,512] recip is 1-lane)
                                zsb = zpool.tile([1, 512], F32R,
                                                 name=f"zs{b}{h}{qc}",
                                                 tag="zs")
                                nc.scalar.copy(zsb[:], pz[:])
                                pzb = ps_x.tile([128, 512], F32,
                                                name=f"pzb{b}{h}{qc}",
                                                tag="pzb")
                                nc.tensor.matmul(pzb[:], onesmr[:], zsb[:],
                                                 start=True, stop=True,
                                                 skip_group_check=True)
                                zb = zpool.tile([128, 512], F32,
                                                name=f"zb{b}{h}{qc}",
                                                tag="zb")
                                nc.vector.reciprocal(zb[:], pzb[:])
                                ao = aop.tile([128, 512], BF,
                                              name=f"ao{b}{h}{qc}",
                                              tag="ao")
                                nc.vector.tensor_mul(ao[:], po[:], zb[:])
                                nc.gpsimd.dma_start(
                                    out=agin[b][qc][h * 128:(h + 1) * 128,
                                                    :],
                                    in_=ao[:])
                                wo_quant_some(3)
                            nc.gpsimd.collective_compute(
                                "AllGather", ALU.bypass, replica_groups=RG,
                                ins=[agin[b][qc][:].opt()],
                                outs=[agout[b][qc][:].opt()])

                qsc2.release()
                wop2.release()

                # ---- phase 3: o_proj, per (batch, qchunk) for overlap ----
                with tc.tile_pool(name="a3", bufs=3) as a3, \
                     tc.tile_pool(name="o3", bufs=2) as o3, \
                     tc.tile_pool(name="p3", bufs=4, space="PSUM") as p3:
                    for ch in range(NTC):
                        b, q2 = ch // 2, ch % 2
                        at = a3.tile([128, NKT * 512], BF, name=f"at{ch}",
                                     tag="at")
                        nc.sync.dma_start(out=at[:],
                                          in_=tiled(agout[b][q2][:, :]))
                        for tt in range(4):
                            pout = p3.tile([128, 512], F32,
                                           name=f"po3_{ch}{tt}", tag="pout")
                            for kt in range(NKT):
                                nc.tensor.matmul(
                                    pout[:],
                                    at[:, kt * 512 + tt * 128:
                                       kt * 512 + (tt + 1) * 128],
                                    wo_sb[:, kt * MSH:(kt + 1) * MSH],
                                    start=(kt == 0), stop=(kt == NKT - 1),
                                    skip_group_check=True)
                            osb = o3.tile([128, 512], F32,
                                          name=f"osb{ch}{tt}", tag="osb")
                            nc.scalar.activation(osb[:], pout[:], ACTF.Copy,
                                                 scale=GO)
                            nc.sync.dma_start(
                                out=outN[ch * 512 + tt * 128:
                                         ch * 512 + (tt + 1) * 128, :],
                                in_=osb[:])
                vnatp.release()
                qkvp.release()
                w3.release()
                gacc.release()

    nc.compile()
    return nc


def kernel(hidden_states, Wq, Wk, Wv, Wo, attention_mask, position_ids):
    from concourse.bass_utils import run_bass_kernel_spmd
    from concourse.bass_interp import get_hw_module

    hs = np.ascontiguousarray(np.asarray(hidden_states, dtype=np.float32))
    Wq = np.asarray(Wq, dtype=np.float32)
    Wk = np.asarray(Wk, dtype=np.float32)
    Wv = np.asarray(Wv, dtype=np.float32)
    Wo = np.asarray(Wo, dtype=np.float32)
    mask = np.asarray(attention_mask, dtype=np.float32)
    posf = np.ascontiguousarray(
        np.asarray(position_ids).reshape(1, T).astype(np.float32))

    status, blk_idx, packed = _classify_mask(mask)
    n_blk = packed.shape[0] // 128
    assert n_blk <= 16, "too many distinct mask blocks"

    key = (status.tobytes(), tuple(sorted(blk_idx.items())), n_blk)
    if key not in _cache:
        nc = _build(status, blk_idx, n_blk)
        nc.m = get_hw_module(nc.m)
        _cache[key] = nc
    nc = _cache[key]

    xT = np.ascontiguousarray(hs.reshape(T, H).T.astype(BF16NP))
    in_maps = []
    for c in range(NCORES):
        in_maps.append({
            "xT": xT,
            "wqT": np.ascontiguousarray(
                Wq[c * QH * HD:(c + 1) * QH * HD, :].T),
            "wkT": np.ascontiguousarray(Wk[c * HD:(c + 1) * HD, :].T),
            "wvT": np.ascontiguousarray(Wv[c * HD:(c + 1) * HD, :].T),
            "woT": np.ascontiguousarray(Wo[c * MSH:(c + 1) * MSH, :].T),
            "maskP": packed,
            "pos": posf,
        })
    res = run_bass_kernel_spmd(nc, in_maps, core_ids=list(range(NCORES)),
                               trace=bool(os.environ.get("BITNET_TRACE")))
    global last_exec_time_ns
    last_exec_time_ns = res.exec_time_ns
    out = np.concatenate(
        [res.results[c]["outN"] for c in range(NCORES)], axis=1)  # (T, MSH*8)
    return np.ascontiguousarray(out).reshape(B, S, H).astype(np.float32)
